# revision 15
# baseline (speedup 1.0000x reference)
"""Trainium2 Bass kernel for nn_AtlasLayerBlock_65395172049082.

- Data-parallel over batch: core c computes batch c % 4 fully (2x redundant).
- Sequence tensors in transposed [d, t] layout on device; host transposes.
- dwconv folded into 3 time-shifted projections (host-prescaled weights).
- Newton-Schulz in rank-192 factor space: X_t = K^T W_t,
  W <- a W + (b M1 + c M1^2) W,  M1 = (W W^T)(K K^T).
- bf16 matmul operands, fp32 accumulation and fp32 memory state M.
- Fully static program (no hardware loops).
"""
import os
import numpy as np
import ml_dtypes

import concourse.bass as bass
import concourse.mybir as mybir
import concourse.tile as tile
from concourse.bass_utils import run_bass_kernel_spmd
from concourse.masks import make_identity

F32 = mybir.dt.float32
BF16 = mybir.dt.bfloat16
AF = mybir.ActivationFunctionType
OP = mybir.AluOpType

B, S, D = 4, 4096, 1024
CHUNK, WINDOW, NS_STEPS, EPS = 64, 128, 5, 1e-6
A_NS, B_NS, C_NS = 3.4445, -4.7750, 2.0315
NCH = S // CHUNK
CTX = WINDOW + CHUNK        # 192
TBLK = 512
NBLK = S // TBLK
DP = D // 128
TT = (128, 64)              # 192 split into t-subtiles

_NC_CACHE = {}


def build():
    nc = bass.Bass("TRN2", num_devices=8)
    dd = nc.declare_dram_parameter
    xT = dd("xT", [D, S], BF16, isOutput=False)
    M0 = dd("M0", [D, D], F32, isOutput=False)
    buf_kT = dd("buf_kT", [D, WINDOW], BF16, isOutput=False)
    buf_v = dd("buf_v", [WINDOW, D], BF16, isOutput=False)
    wk = [dd(f"WkT{r}", [D, D], BF16, isOutput=False) for r in range(3)]
    wq = [dd(f"WqT{r}", [D, D], BF16, isOutput=False) for r in range(3)]
    wv = dd("WvT", [D, D], BF16, isOutput=False)
    wg = dd("WgT", [D, 3 * D], BF16, isOutput=False)
    wb = dd("WbT", [D, D], BF16, isOutput=False)
    conv_k_b = dd("conv_k_b", [D, 1], F32, isOutput=False)
    conv_q_b = dd("conv_q_b", [D, 1], F32, isOutput=False)
    w_in = dd("norm_in_w", [D, 1], F32, isOutput=False)
    w_kq = dd("norm_kq_w", [D, 1], F32, isOutput=False)
    w_out = dd("norm_out_w", [D, 1], F32, isOutput=False)
    outT = dd("outT", [D, S], F32, isOutput=True)
    M_out = dd("M_out", [D, D], F32, isOutput=True)
    bkT_out = dd("bkT", [D, WINDOW], F32, isOutput=True)
    bv_out = dd("bv", [WINDOW, D], F32, isOutput=True)

    xT3 = xT.rearrange("(dp p) t -> p dp t", p=128)
    outT3 = outT.rearrange("(dp p) t -> p dp t", p=128)
    M03 = M0.rearrange("(dp p) j -> p dp j", p=128)
    Mo3 = M_out.rearrange("(dp p) j -> p dp j", p=128)
    wv3 = wv.rearrange("(dp p) o -> p dp o", p=128)
    wg3 = wg.rearrange("(dp p) o -> p dp o", p=128)
    wb3 = wb.rearrange("(dp p) o -> p dp o", p=128)
    wk3 = [w.rearrange("(dp p) o -> p dp o", p=128) for w in wk]
    wq3 = [w.rearrange("(dp p) o -> p dp o", p=128) for w in wq]
    ckb2 = conv_k_b.rearrange("(dp p) x -> p (dp x)", p=128)
    cqb2 = conv_q_b.rearrange("(dp p) x -> p (dp x)", p=128)
    win2 = w_in.rearrange("(dp p) x -> p (dp x)", p=128)
    wkq2 = w_kq.rearrange("(dp p) x -> p (dp x)", p=128)
    wout2 = w_out.rearrange("(dp p) x -> p (dp x)", p=128)
    bkT3 = bkT_out.rearrange("(dp p) t -> p dp t", p=128)

    with tile.TileContext(nc) as tc:
        with (
            tc.tile_pool(name="cst", bufs=1) as cst,
            tc.tile_pool(name="dram", bufs=1, space="DRAM") as dram,
            tc.tile_pool(name="persist", bufs=1) as persist,
        ):
            kT_ext = dram.tile([128, DP, WINDOW + S], BF16)
            xn_d = dram.tile([128, DP, S + 2], BF16)
            v_ext = dram.tile([WINDOW + S, D], BF16)
            qT_d = dram.tile([128, DP, S], BF16)
            bgT_d = dram.tile([128, DP, S], BF16)

            idf = cst.tile([128, 128], F32)
            make_identity(nc, idf[:])
            idb = cst.tile([128, 128], BF16)
            nc.vector.tensor_copy(idb[:], idf[:])
            aIb = cst.tile([128, 128], BF16)
            nc.vector.tensor_scalar_mul(aIb[:], idf[:], A_NS)
            aIf = cst.tile([128, 128], F32)
            nc.vector.tensor_scalar_mul(aIf[:], idf[:], A_NS)
            ones_col_b = cst.tile([128, 1], BF16)
            nc.gpsimd.memset(ones_col_b[:], 1.0)
            ones_row_b = cst.tile([1, 128], BF16)
            nc.gpsimd.memset(ones_row_b[:], 1.0)
            ones_row_f = cst.tile([1, 128], F32)
            nc.gpsimd.memset(ones_row_f[:], 1.0)
            win_sb = cst.tile([128, DP], F32)
            nc.sync.dma_start(win_sb[:], win2[:])
            wkq_sb = cst.tile([128, DP], F32)
            nc.sync.dma_start(wkq_sb[:], wkq2[:])
            wout_sb = cst.tile([128, DP], F32)
            nc.sync.dma_start(wout_sb[:], wout2[:])
            eps_col = cst.tile([128, 1], F32)
            nc.gpsimd.memset(eps_col[:], EPS)
            eps7_col = cst.tile([128, 1], F32)
            nc.gpsimd.memset(eps7_col[:], 1e-7)
            ckb_sb = cst.tile([128, DP], F32)
            nc.sync.dma_start(ckb_sb[:], ckb2[:])
            cqb_sb = cst.tile([128, DP], F32)
            nc.sync.dma_start(cqb_sb[:], cqb2[:])

            etaM = persist.tile([128, DP, NCH], F32)
            alM = persist.tile([128, DP, NCH], F32)

            nc.sync.dma_start(kT_ext[:, :, 0:WINDOW],
                              buf_kT.rearrange("(dp p) t -> p dp t", p=128))
            nc.sync.dma_start(v_ext[0:WINDOW, :], buf_v[:])

            # ================= PHASE 1 =================
            with (
                tc.tile_pool(name="p1", bufs=2) as p1,
                tc.tile_pool(name="p1x", bufs=1) as p1x,
                tc.tile_pool(name="p1w", bufs=2) as p1w,
                tc.tile_pool(name="p1ps", bufs=4, space="PSUM") as p1ps,
                tc.tile_pool(name="p1st", bufs=2, space="PSUM") as p1st,
            ):
                # normalized input with halo in DRAM: col c = token c-1
                bv_f32 = p1x.tile([128, D], F32)
                bk_f32 = p1x.tile([128, DP, WINDOW], F32)
                zc = p1x.tile([128, DP, 1], BF16)
                nc.gpsimd.memset(zc[:], 0.0)
                nc.sync.dma_start(xn_d[:, :, 0:1], zc[:])
                nc.sync.dma_start(xn_d[:, :, S + 1:S + 2], zc[:])

                def rstats_inv(src_tile, n):
                    sq = p1.tile([128, DP, n], BF16, tag="sq", name="sq")
                    nc.scalar.activation(sq[:], src_tile[:], AF.Square)
                    ss = p1st.tile([1, TBLK], F32, tag="ss", name="ss")[:, :n]
                    for dp in range(DP):
                        nc.tensor.matmul(ss, ones_col_b[:], sq[:, dp, :],
                                         start=(dp == 0), stop=(dp == DP - 1))
                    inv = p1.tile([1, n], F32, tag="inv", name="inv")
                    nc.scalar.activation(inv[:], ss, AF.Sqrt,
                                         bias=eps_col[:1, :], scale=1.0 / D)
                    nc.vector.reciprocal(inv[:], inv[:])
                    invb = p1.tile([1, n], BF16, tag="invb", name="invb")
                    nc.vector.tensor_copy(invb[:], inv[:])
                    ibc = p1st.tile([128, TBLK], F32, tag="ibc", name="ibc")[:, :n]
                    nc.tensor.matmul(ibc, ones_row_b[:], invb[:],
                                     start=True, stop=True)
                    return ibc  # [128, n] f32 psum, row-broadcast of inv_rms

                for blk in range(NBLK):
                    t0 = blk * TBLK
                    xb = p1.tile([128, DP, TBLK], BF16, tag="xb", name="xb")
                    nc.sync.dma_start(xb[:], xT3[:, :, t0:t0 + TBLK])
                    ibc = rstats_inv(xb, TBLK)
                    xnw = p1.tile([128, DP, TBLK], BF16, tag="xnw", name="xnw")
                    for dp in range(DP):
                        nc.vector.tensor_tensor(xnw[:, dp, :], xb[:, dp, :],
                                                ibc, OP.mult)
                        nc.vector.tensor_tensor(
                            xnw[:, dp, :], xnw[:, dp, :],
                            win_sb[:, dp, None].to_broadcast((128, TBLK)), OP.mult)
                    nc.sync.dma_start(xn_d[:, :, 1 + t0:1 + t0 + TBLK], xnw[:])

                for blk in range(NBLK):
                    t0 = blk * TBLK
                    xnb = p1.tile([128, DP, TBLK + 2], BF16, tag="xnb", name="xnb")
                    nc.sync.dma_start(xnb[:], xn_d[:, :, t0:t0 + TBLK + 2])
                    # ---- k / q with folded conv ----
                    for (wts, bias_sb, is_k) in (
                        (wk3, ckb_sb, True), (wq3, cqb_sb, False),
                    ):
                        kconv = p1.tile([128, DP, TBLK], BF16, tag="kconv", name="kconv")
                        for ot in range(DP):
                            ps = p1ps.tile([128, TBLK], F32, tag="mm", name="mm")
                            first = True
                            for r in range(3):
                                wt = p1w.tile([128, DP, 128], BF16, tag="wt", name="wt")
                                nc.sync.dma_start(
                                    wt[:], wts[r][:, :, ot * 128:(ot + 1) * 128])
                                for dp in range(DP):
                                    nc.tensor.matmul(
                                        ps[:], wt[:, dp, :],
                                        xnb[:, dp, r:r + TBLK],
                                        start=first,
                                        stop=(r == 2 and dp == DP - 1))
                                    first = False
                            nc.scalar.activation(kconv[:, ot, :], ps[:], AF.Silu,
                                                 bias=bias_sb[:, ot:ot + 1])
                        ibc = rstats_inv(kconv, TBLK)
                        kfin = p1.tile([128, DP, TBLK], BF16, tag="kfin", name="kfin")
                        for dp in range(DP):
                            nc.vector.tensor_tensor(kfin[:, dp, :],
                                                    kconv[:, dp, :], ibc, OP.mult)
                            nc.vector.tensor_tensor(
                                kfin[:, dp, :], kfin[:, dp, :],
                                wkq_sb[:, dp, None].to_broadcast((128, TBLK)),
                                OP.mult)
                        if is_k:
                            nc.sync.dma_start(
                                kT_ext[:, :, WINDOW + t0:WINDOW + t0 + TBLK],
                                kfin[:])
                            if blk == NBLK - 1:
                                for dp in range(DP):
                                    nc.vector.tensor_copy(
                                        bk_f32[:, dp, :],
                                        kfin[:, dp, TBLK - WINDOW:])
                        else:
                            nc.sync.dma_start(qT_d[:, :, t0:t0 + TBLK], kfin[:])

                    # ---- gates ----
                    gam = p1.tile([128, DP, TBLK], BF16, tag="gam", name="gam")
                    for gi in range(3):  # gamma, eta, alpha
                        for ot in range(DP):
                            ps = p1ps.tile([128, TBLK], F32, tag="mm", name="mm")
                            wt = p1w.tile([128, DP, 128], BF16, tag="wt", name="wt")
                            nc.sync.dma_start(
                                wt[:],
                                wg3[:, :, gi * D + ot * 128:gi * D + (ot + 1) * 128])
                            for dp in range(DP):
                                nc.tensor.matmul(
                                    ps[:], wt[:, dp, :],
                                    xnb[:, dp, 1:1 + TBLK],
                                    start=(dp == 0), stop=(dp == DP - 1))
                            if gi == 0:
                                nc.scalar.activation(gam[:, ot, :], ps[:], AF.Silu)
                            else:
                                sil = p1.tile([128, TBLK], F32, tag="sil", name="sil")
                                nc.scalar.activation(sil[:], ps[:], AF.Silu)
                                dst = etaM if gi == 1 else alM
                                nc.vector.tensor_reduce(
                                    dst[:, ot, blk * 8:(blk + 1) * 8],
                                    sil[:].rearrange("p (c x) -> p c x", x=CHUNK),
                                    mybir.AxisListType.X, OP.add)

                    # ---- bypass -> bg = silu(byp) * gamma * w_out ----
                    bg = p1.tile([128, DP, TBLK], BF16, tag="bg", name="bg")
                    for ot in range(DP):
                        ps = p1ps.tile([128, TBLK], F32, tag="mm", name="mm")
                        wt = p1w.tile([128, DP, 128], BF16, tag="wt", name="wt")
                        nc.sync.dma_start(wt[:], wb3[:, :, ot * 128:(ot + 1) * 128])
                        for dp in range(DP):
                            nc.tensor.matmul(ps[:], wt[:, dp, :],
                                             xnb[:, dp, 1:1 + TBLK],
                                             start=(dp == 0), stop=(dp == DP - 1))
                        sil = p1.tile([128, TBLK], F32, tag="sil", name="sil")
                        nc.scalar.activation(sil[:], ps[:], AF.Silu)
                        nc.vector.tensor_tensor(bg[:, ot, :], sil[:],
                                                gam[:, ot, :], OP.mult)
                        nc.vector.tensor_tensor(
                            bg[:, ot, :], bg[:, ot, :],
                            wout_sb[:, ot, None].to_broadcast((128, TBLK)),
                            OP.mult)
                    nc.sync.dma_start(bgT_d[:, :, t0:t0 + TBLK], bg[:])

                    # ---- v (natural layout) ----
                    vts = [p1.tile([128, D], BF16, tag=f"vt{tt}", name=f"vt{tt}")
                           for tt in range(TBLK // 128)]
                    for nj in range(2):
                        wvt = p1w.tile([128, DP, TBLK], BF16, tag="wvt", name="wvt")
                        nc.sync.dma_start(wvt[:],
                                          wv3[:, :, nj * TBLK:(nj + 1) * TBLK])
                        for tt in range(TBLK // 128):
                            ps = p1ps.tile([128, TBLK], F32, tag="mm", name="mm")
                            for dp in range(DP):
                                nc.tensor.matmul(
                                    ps[:],
                                    xnb[:, dp,
                                        1 + tt * 128:1 + (tt + 1) * 128],
                                    wvt[:, dp, :],
                                    start=(dp == 0), stop=(dp == DP - 1))
                            nc.scalar.activation(
                                vts[tt][:, nj * TBLK:(nj + 1) * TBLK], ps[:],
                                AF.Silu)
                    for tt in range(TBLK // 128):
                        row0 = WINDOW + t0 + tt * 128
                        nc.sync.dma_start(v_ext[row0:row0 + 128, :], vts[tt][:])
                    if blk == NBLK - 1:
                        nc.vector.tensor_copy(bv_f32[:], vts[TBLK // 128 - 1][:])

                nc.sync.dma_start(bkT3[:], bk_f32[:])
                nc.sync.dma_start(bv_out[:], bv_f32[:])

            # ================= PHASE 2 =================
            with (
                tc.tile_pool(name="p2", bufs=2) as p2,
                tc.tile_pool(name="p2m", bufs=1) as p2m,
                tc.tile_pool(name="mmps", bufs=3, space="PSUM") as mmps,
                tc.tile_pool(name="tpps", bufs=3, space="PSUM") as tpps,
                tc.tile_pool(name="aux", bufs=2, space="PSUM") as aux,
            ):
                M_sb = p2m.tile([128, DP, D], F32)
                M_bf = p2m.tile([128, DP, D], BF16)
                nc.sync.dma_start(M_sb[:], M03[:])
                for dp in range(DP):
                    nc.vector.tensor_copy(M_bf[:, dp, :], M_sb[:, dp, :])
                def transpose_to(dst_ap, src_ap, pw, fw, ident):
                    dt = BF16 if ident is idb else F32
                    ps = tpps.tile([128, 128], dt, tag="tp", name="tp")[:fw, :pw]
                    nc.tensor.transpose(ps, src_ap, ident[:pw, :pw])
                    nc.vector.tensor_copy(dst_ap, ps)

                for ci in range(NCH):
                    c0 = ci * CHUNK
                    ctxT = p2.tile([128, DP, CTX], BF16, tag="ctxT", name="ctxT")
                    nc.sync.dma_start(ctxT[:], kT_ext[:, :, c0:c0 + CTX])
                    ctxv = [p2.tile([tw, D], BF16, tag=f"ctxv{ti}", name=f"ctxv{ti}")
                            for ti, tw in enumerate(TT)]
                    nc.sync.dma_start(ctxv[0][:], v_ext[c0:c0 + 128, :])
                    nc.sync.dma_start(ctxv[1][:], v_ext[c0 + 128:c0 + CTX, :])
                    qch = p2.tile([128, DP, CHUNK], BF16, tag="qch", name="qch")
                    nc.sync.dma_start(qch[:], qT_d[:, :, c0:c0 + CHUNK])
                    bgch = p2.tile([128, DP, CHUNK], BF16, tag="bgch", name="bgch")
                    nc.sync.dma_start(bgch[:], bgT_d[:, :, c0:c0 + CHUNK])

                    # err = ctx_k @ M - ctx_v
                    errt = [p2.tile([tw, D], BF16, tag=f"errt{ti}", name=f"errt{ti}")
                            for ti, tw in enumerate(TT)]
                    for ti, tw in enumerate(TT):
                        toff = ti * 128
                        for nj in range(2):
                            ps = mmps.tile([128, 512], F32, tag="mm", name="mm")[:tw]
                            for dp in range(DP):
                                nc.tensor.matmul(
                                    ps, ctxT[:, dp, toff:toff + tw],
                                    M_bf[:, dp, nj * 512:(nj + 1) * 512],
                                    start=(dp == 0), stop=(dp == DP - 1))
                            nc.vector.tensor_tensor(
                                errt[ti][:, nj * 512:(nj + 1) * 512], ps,
                                ctxv[ti][:, nj * 512:(nj + 1) * 512], OP.subtract)

                    # S = K K^T
                    Sg = [p2.tile([tw, CTX], BF16, tag=f"Sg{ti}", name=f"Sg{ti}")
                          for ti, tw in enumerate(TT)]
                    Sf = [p2.tile([tw, CTX], F32, tag=f"Sf{ti}", name=f"Sf{ti}")
                          for ti, tw in enumerate(TT)]
                    for ti, tw in enumerate(TT):
                        toff = ti * 128
                        ps = mmps.tile([128, 512], F32, tag="mm", name="mm")[:tw, :CTX]
                        for dp in range(DP):
                            nc.tensor.matmul(ps, ctxT[:, dp, toff:toff + tw],
                                             ctxT[:, dp, :],
                                             start=(dp == 0), stop=(dp == DP - 1))
                        nc.vector.tensor_copy(Sg[ti][:], ps)
                        nc.vector.tensor_copy(Sf[ti][:], ps)

                    # ET
                    ET = p2.tile([128, DP, CTX], BF16, tag="ET", name="ET")
                    for ti, tw in enumerate(TT):
                        for jt in range(DP):
                            transpose_to(ET[:, jt, ti * 128:ti * 128 + tw],
                                         errt[ti][:, jt * 128:(jt + 1) * 128],
                                         tw, 128, idb)

                    # ||G||^2 = sum(S * (E E^T))
                    zsum = p2.tile([128, 2], F32, tag="zsum", name="zsum")
                    nc.gpsimd.memset(zsum[:], 0.0)
                    for ti, tw in enumerate(TT):
                        toff = ti * 128
                        ps = mmps.tile([128, 512], F32, tag="mm", name="mm")[:tw, :CTX]
                        for jt in range(DP):
                            nc.tensor.matmul(ps, ET[:, jt, toff:toff + tw],
                                             ET[:, jt, :],
                                             start=(jt == 0), stop=(jt == DP - 1))
                        z = p2.tile([tw, CTX], F32, tag=f"z{ti}", name=f"z{ti}")
                        nc.vector.tensor_tensor(z[:], ps, Sf[ti][:], OP.mult)
                        nc.vector.tensor_reduce(zsum[:tw, ti:ti + 1], z[:],
                                                mybir.AxisListType.X, OP.add)
                    zsb = p2.tile([128, 2], BF16, tag="zsb", name="zsb")
                    nc.vector.tensor_copy(zsb[:], zsum[:])
                    g2ps = aux.tile([128, 512], F32, tag="aux", name="aux")[:1, :2]
                    nc.tensor.matmul(g2ps, ones_col_b[:], zsb[:],
                                     start=True, stop=True)
                    g2 = p2.tile([1, 1], F32, tag="g2", name="g2")
                    nc.vector.tensor_reduce(g2[:], g2ps, mybir.AxisListType.X,
                                            OP.add)
                    nc.scalar.activation(g2[:], g2[:], AF.Sqrt)
                    nc.scalar.add(g2[:], g2[:], eps7_col[:1, :])
                    nc.vector.reciprocal(g2[:], g2[:])
                    g2b = p2.tile([1, 1], BF16, tag="g2b", name="g2b")
                    nc.vector.tensor_copy(g2b[:], g2[:])
                    gcol = aux.tile([128, 512], F32, tag="aux", name="aux")[:, :1]
                    nc.tensor.matmul(gcol, ones_row_b[:], g2b[:],
                                     start=True, stop=True)

                    # W0 / WT0
                    Wc = [p2.tile([tw, D], F32, tag=f"W{ti}", name=f"W{ti}")
                          for ti, tw in enumerate(TT)]
                    for ti, tw in enumerate(TT):
                        nc.vector.tensor_tensor(
                            Wc[ti][:], errt[ti][:],
                            gcol[:tw].to_broadcast((tw, D)), OP.mult)
                    WT = p2.tile([128, DP, CTX], F32, tag="WT", name="WT")
                    for ti, tw in enumerate(TT):
                        for jt in range(DP):
                            transpose_to(WT[:, jt, ti * 128:ti * 128 + tw],
                                         Wc[ti][:, jt * 128:(jt + 1) * 128],
                                         tw, 128, idf)

                    # ---- NS steps ----
                    for step in range(NS_STEPS):
                        P_ = [p2.tile([tw, CTX], F32, tag=f"P{ti}", name=f"P{ti}")
                              for ti, tw in enumerate(TT)]
                        for ti, tw in enumerate(TT):
                            toff = ti * 128
                            ps = mmps.tile([128, 512], F32, tag="mm", name="mm")[:tw, :CTX]
                            for jt in range(DP):
                                nc.tensor.matmul(ps, WT[:, jt, toff:toff + tw],
                                                 WT[:, jt, :],
                                                 start=(jt == 0),
                                                 stop=(jt == DP - 1))
                            nc.vector.tensor_copy(P_[ti][:], ps)
                        M1b = [p2.tile([tw, CTX], F32, tag=f"M1b{ti}", name=f"M1b{ti}")
                               for ti, tw in enumerate(TT)]
                        for ti, tw in enumerate(TT):
                            toff = ti * 128
                            ps = mmps.tile([128, 512], F32, tag="mm", name="mm")[:tw, :CTX]
                            for ct in range(2):
                                nc.tensor.matmul(ps, P_[ct][:, toff:toff + tw],
                                                 Sf[ct][:],
                                                 start=(ct == 0), stop=(ct == 1))
                            nc.vector.tensor_scalar_mul(M1b[ti][:], ps, B_NS)
                        M1bT = [p2.tile([tw, CTX], F32, tag=f"M1bT{ti}", name=f"M1bT{ti}")
                                for ti, tw in enumerate(TT)]
                        for ti, tw in enumerate(TT):
                            for ct, cw in enumerate(TT):
                                transpose_to(
                                    M1bT[ti][:, ct * 128:ct * 128 + cw],
                                    M1b[ct][:, ti * 128:ti * 128 + tw],
                                    cw, tw, idf)
                        M4 = [p2.tile([tw, CTX], F32, tag=f"M4{ti}", name=f"M4{ti}")
                              for ti, tw in enumerate(TT)]
                        cb2 = C_NS / (B_NS * B_NS)
                        for ti, tw in enumerate(TT):
                            toff = ti * 128
                            ps = mmps.tile([128, 512], F32, tag="mm", name="mm")[:tw, :CTX]
                            for ct in range(2):
                                nc.tensor.matmul(ps, M1bT[ct][:, toff:toff + tw],
                                                 M1b[ct][:],
                                                 start=(ct == 0), stop=(ct == 1))
                            t4 = p2.tile([tw, CTX], F32, tag=f"t4{ti}", name=f"t4{ti}")
                            nc.vector.tensor_scalar_mul(t4[:], ps, cb2)
                            nc.vector.tensor_tensor(M4[ti][:], t4[:], M1b[ti][:],
                                                    OP.add)
                        M4T = [p2.tile([tw, CTX], F32, tag=f"M4T{ti}", name=f"M4T{ti}")
                               for ti, tw in enumerate(TT)]
                        for ti, tw in enumerate(TT):
                            for ct, cw in enumerate(TT):
                                transpose_to(
                                    M4T[ti][:, ct * 128:ct * 128 + cw],
                                    M4[ct][:, ti * 128:ti * 128 + tw],
                                    cw, tw, idf)
                        Wn = [p2.tile([tw, D], F32, tag=f"W{ti}", name=f"W{ti}")
                              for ti, tw in enumerate(TT)]
                        for ti, tw in enumerate(TT):
                            toff = ti * 128
                            for nj in range(2):
                                ps = mmps.tile([128, 512], F32, tag="mm", name="mm")[:tw]
                                nc.tensor.matmul(ps, M4T[0][:, toff:toff + tw],
                                                 Wc[0][:, nj * 512:(nj + 1) * 512],
                                                 start=True, stop=False)
                                nc.tensor.matmul(ps, M4T[1][:, toff:toff + tw],
                                                 Wc[1][:, nj * 512:(nj + 1) * 512],
                                                 start=False, stop=False)
                                nc.tensor.matmul(ps, aIf[:tw, :tw],
                                                 Wc[ti][:, nj * 512:(nj + 1) * 512],
                                                 start=False, stop=True)
                                nc.vector.tensor_copy(
                                    Wn[ti][:, nj * 512:(nj + 1) * 512], ps)
                        Wc = Wn
                        if step < NS_STEPS - 1:
                            WT = p2.tile([128, DP, CTX], F32, tag="WT", name="WT")
                            for ti, tw in enumerate(TT):
                                for jt in range(DP):
                                    transpose_to(
                                        WT[:, jt, ti * 128:ti * 128 + tw],
                                        Wc[ti][:, jt * 128:(jt + 1) * 128],
                                        tw, 128, idf)

                    # bf16 copy of W5 for og
                    W5b = [p2.tile([tw, D], BF16, tag=f"W5b{ti}", name=f"W5b{ti}")
                           for ti, tw in enumerate(TT)]
                    for ti, tw in enumerate(TT):
                        nc.vector.tensor_copy(W5b[ti][:], Wc[ti][:])
                    # ctx_k natural
                    ctxN = [p2.tile([tw, D], BF16, tag=f"ctxN{ti}", name=f"ctxN{ti}")
                            for ti, tw in enumerate(TT)]
                    for ti, tw in enumerate(TT):
                        for it in range(DP):
                            transpose_to(ctxN[ti][:, it * 128:(it + 1) * 128],
                                         ctxT[:, it, ti * 128:ti * 128 + tw],
                                         128, tw, idb)

                    # eta/alpha rows -> [1, D] via vec transposes, then bcast
                    ebc = p2.tile([128, D], F32, tag="ebc", name="ebc")
                    abc = p2.tile([128, D], F32, tag="abc", name="abc")
                    for (src, dst) in ((etaM, ebc), (alM, abc)):
                        row = p2.tile([1, D], F32, tag="row", name="row")
                        for dp in range(DP):
                            ps = tpps.tile([128, 128], F32, tag="tp", name="tp")[:1, :128]
                            nc.tensor.transpose(ps, src[:, dp, ci:ci + 1],
                                                idf[:])
                            nc.vector.tensor_scalar_mul(
                                row[:, dp * 128:(dp + 1) * 128], ps,
                                1.0 / CHUNK)
                        rowb = p2.tile([1, D], BF16, tag="rowb", name="rowb")
                        nc.vector.tensor_copy(rowb[:], row[:])
                        for nj in range(2):
                            ps = aux.tile([128, 512], F32, tag="aux", name="aux")
                            nc.tensor.matmul(
                                ps[:], ones_row_b[:],
                                rowb[:, nj * 512:(nj + 1) * 512],
                                start=True, stop=True)
                            nc.vector.tensor_copy(dst[:, nj * 512:(nj + 1) * 512],
                                                  ps[:])

                    # og = K^T W5 (pieces) and M update
                    for it in range(DP):
                        for nj in range(2):
                            sl = slice(nj * 512, (nj + 1) * 512)
                            ps = mmps.tile([128, 512], F32, tag="mm", name="mm")
                            nc.tensor.matmul(ps[:],
                                             ctxN[0][:, it * 128:(it + 1) * 128],
                                             W5b[0][:, sl], start=True, stop=False)
                            nc.tensor.matmul(ps[:],
                                             ctxN[1][:, it * 128:(it + 1) * 128],
                                             W5b[1][:, sl], start=False, stop=True)
                            t5 = p2.tile([128, 512], F32, tag="t5", name="t5")
                            nc.vector.tensor_tensor(t5[:], ps[:], ebc[:, sl],
                                                    OP.mult)
                            nc.vector.tensor_tensor(M_sb[:, it, sl],
                                                    M_sb[:, it, sl],
                                                    abc[:, sl], OP.mult)
                            nc.vector.tensor_tensor(M_sb[:, it, sl],
                                                    M_sb[:, it, sl], t5[:],
                                                    OP.subtract)
                        nc.vector.tensor_copy(M_bf[:, it, :], M_sb[:, it, :])

                    # c_out + fused epilogue
                    co = p2.tile([128, DP, CHUNK], BF16, tag="co", name="co")
                    for mj in range(DP):
                        ps = mmps.tile([128, 512], F32, tag="mm", name="mm")[:, :CHUNK]
                        for dp in range(DP):
                            nc.tensor.matmul(ps,
                                             M_bf[:, dp, mj * 128:(mj + 1) * 128],
                                             qch[:, dp, :],
                                             start=(dp == 0), stop=(dp == DP - 1))
                        nc.vector.tensor_copy(co[:, mj, :], ps)
                    csq = p2.tile([128, DP, CHUNK], BF16, tag="csq", name="csq")
                    nc.scalar.activation(csq[:], co[:], AF.Square)
                    ssps = aux.tile([128, 512], F32, tag="aux", name="aux")[:1, :CHUNK]
                    for dp in range(DP):
                        nc.tensor.matmul(ssps, ones_col_b[:], csq[:, dp, :],
                                         start=(dp == 0), stop=(dp == DP - 1))
                    oinv = p2.tile([1, CHUNK], F32, tag="oinv", name="oinv")
                    nc.scalar.activation(oinv[:], ssps, AF.Sqrt,
                                         bias=eps_col[:1, :], scale=1.0 / D)
                    nc.vector.reciprocal(oinv[:], oinv[:])
                    oinvb = p2.tile([1, CHUNK], BF16, tag="oinvb", name="oinvb")
                    nc.vector.tensor_copy(oinvb[:], oinv[:])
                    oibc = aux.tile([128, 512], F32, tag="aux", name="aux")[:, :CHUNK]
                    nc.tensor.matmul(oibc, ones_row_b[:], oinvb[:],
                                     start=True, stop=True)
                    oout = p2.tile([128, DP, CHUNK], F32, tag="oout", name="oout")
                    for dp in range(DP):
                        t6 = p2.tile([128, CHUNK], F32, tag="t6", name="t6")
                        nc.vector.tensor_tensor(t6[:], co[:, dp, :], oibc,
                                                OP.mult)
                        nc.vector.tensor_tensor(oout[:, dp, :], t6[:],
                                                bgch[:, dp, :], OP.mult)
                    nc.sync.dma_start(outT3[:, :, c0:c0 + CHUNK], oout[:])

                nc.sync.dma_start(Mo3[:], M_sb[:])
    nc.compile()
    return nc


def _get_nc():
    if "nc" not in _NC_CACHE:
        _NC_CACHE["nc"] = build()
    return _NC_CACHE["nc"]


def kernel(x, mem_state, buf_k, buf_v, norm_in_w, norm_kq_w, norm_out_w,
           Wk, Wq, Wv, Wg, Wb, conv_k_w, conv_k_b, conv_q_w, conv_q_b):
    x = np.asarray(x, np.float32)
    bf = lambda a: np.ascontiguousarray(np.asarray(a, np.float32)).astype(ml_dtypes.bfloat16)
    f32 = lambda a: np.ascontiguousarray(np.asarray(a, np.float32))

    ckw = np.asarray(conv_k_w, np.float32)
    cqw = np.asarray(conv_q_w, np.float32)
    WkTs = [bf(np.asarray(Wk).T * ckw[:, 0, r][None, :]) for r in range(3)]
    WqTs = [bf(np.asarray(Wq).T * cqw[:, 0, r][None, :]) for r in range(3)]
    shared = {
        "WkT0": WkTs[0], "WkT1": WkTs[1], "WkT2": WkTs[2],
        "WqT0": WqTs[0], "WqT1": WqTs[1], "WqT2": WqTs[2],
        "WvT": bf(np.asarray(Wv).T), "WgT": bf(np.asarray(Wg).T),
        "WbT": bf(np.asarray(Wb).T),
        "conv_k_b": f32(conv_k_b)[:, None], "conv_q_b": f32(conv_q_b)[:, None],
        "norm_in_w": f32(norm_in_w)[:, None],
        "norm_kq_w": f32(norm_kq_w)[:, None],
        "norm_out_w": f32(norm_out_w)[:, None],
    }
    in_maps = []
    for c in range(8):
        b = c % B
        m = dict(shared)
        m["xT"] = bf(np.ascontiguousarray(x[b].T))
        m["M0"] = f32(mem_state[b])
        m["buf_kT"] = bf(np.asarray(buf_k)[b].T)
        m["buf_v"] = bf(np.asarray(buf_v)[b])
        in_maps.append(m)

    nc = _get_nc()
    if os.environ.get("ATLAS_TRACE"):
        try:
            r = run_bass_kernel_spmd(nc, in_maps, list(range(8)), trace=True)
            globals()["LAST_EXEC_NS"] = r.exec_time_ns
            res = r.results
        except (ImportError, ModuleNotFoundError):
            import time as _t
            res = run_bass_kernel_spmd(nc, in_maps, list(range(8))).results
            t0 = _t.time()
            res = run_bass_kernel_spmd(nc, in_maps, list(range(8))).results
            globals()["LAST_EXEC_NS"] = int((_t.time() - t0) * 1e9)
    else:
        res = run_bass_kernel_spmd(nc, in_maps, list(range(8))).results

    out = np.stack([res[b]["outT"].T for b in range(B)])
    M = np.stack([res[b]["M_out"] for b in range(B)])
    bk = np.stack([res[b]["bkT"].T for b in range(B)])
    bv = np.stack([res[b]["bv"] for b in range(B)])
    return (out.astype(np.float32), M.astype(np.float32),
            bk.astype(np.float32), bv.astype(np.float32))


# revision 16
# speedup vs baseline: 1.1588x; 1.1588x over previous
"""Trainium2 Bass kernel for nn_AtlasLayerBlock_65395172049082.

- Data-parallel over batch: core c computes batch c % 4 fully (2x redundant).
- Sequence tensors in transposed [d, t] layout on device; host transposes.
- dwconv folded into 3 time-shifted projections (host-prescaled weights).
- Newton-Schulz in rank-192 factor space: X_t = K^T W_t,
  W <- a W + (b M1 + c M1^2) W,  M1 = (W W^T)(K K^T).
- bf16 matmul operands, fp32 accumulation and fp32 memory state M.
- Fully static program (no hardware loops).
"""
import os
import numpy as np
import ml_dtypes

import concourse.bass as bass
import concourse.mybir as mybir
import concourse.tile as tile
from concourse.bass_utils import run_bass_kernel_spmd
from concourse.masks import make_identity

F32 = mybir.dt.float32
BF16 = mybir.dt.bfloat16
AF = mybir.ActivationFunctionType
OP = mybir.AluOpType

B, S, D = 4, 4096, 1024
CHUNK, WINDOW, NS_STEPS, EPS = 64, 128, 5, 1e-6
A_NS, B_NS, C_NS = 3.4445, -4.7750, 2.0315
NCH = S // CHUNK
CTX = WINDOW + CHUNK        # 192
TBLK = 512
NBLK = S // TBLK
DP = D // 128
TT = (128, 64)              # 192 split into t-subtiles

_NC_CACHE = {}


def build():
    nc = bass.Bass("TRN2", num_devices=8)
    dd = nc.declare_dram_parameter
    xT = dd("xT", [D, S], BF16, isOutput=False)
    M0 = dd("M0", [D, D], F32, isOutput=False)
    buf_kT = dd("buf_kT", [D, WINDOW], BF16, isOutput=False)
    buf_v = dd("buf_v", [WINDOW, D], BF16, isOutput=False)
    wk = [dd(f"WkT{r}", [D, D], BF16, isOutput=False) for r in range(3)]
    wq = [dd(f"WqT{r}", [D, D], BF16, isOutput=False) for r in range(3)]
    wv = dd("WvT", [D, D], BF16, isOutput=False)
    wg = dd("WgT", [D, 3 * D], BF16, isOutput=False)
    wb = dd("WbT", [D, D], BF16, isOutput=False)
    conv_k_b = dd("conv_k_b", [D, 1], F32, isOutput=False)
    conv_q_b = dd("conv_q_b", [D, 1], F32, isOutput=False)
    w_in = dd("norm_in_w", [D, 1], F32, isOutput=False)
    w_kq = dd("norm_kq_w", [D, 1], F32, isOutput=False)
    w_out = dd("norm_out_w", [D, 1], F32, isOutput=False)
    outT = dd("outT", [D, S], F32, isOutput=True)
    M_out = dd("M_out", [D, D], F32, isOutput=True)
    bkT_out = dd("bkT", [D, WINDOW], F32, isOutput=True)
    bv_out = dd("bv", [WINDOW, D], F32, isOutput=True)

    xT3 = xT.rearrange("(dp p) t -> p dp t", p=128)
    outT3 = outT.rearrange("(dp p) t -> p dp t", p=128)
    M03 = M0.rearrange("(dp p) j -> p dp j", p=128)
    Mo3 = M_out.rearrange("(dp p) j -> p dp j", p=128)
    wv3 = wv.rearrange("(dp p) o -> p dp o", p=128)
    wg3 = wg.rearrange("(dp p) o -> p dp o", p=128)
    wb3 = wb.rearrange("(dp p) o -> p dp o", p=128)
    wk3 = [w.rearrange("(dp p) o -> p dp o", p=128) for w in wk]
    wq3 = [w.rearrange("(dp p) o -> p dp o", p=128) for w in wq]
    ckb2 = conv_k_b.rearrange("(dp p) x -> p (dp x)", p=128)
    cqb2 = conv_q_b.rearrange("(dp p) x -> p (dp x)", p=128)
    win2 = w_in.rearrange("(dp p) x -> p (dp x)", p=128)
    wkq2 = w_kq.rearrange("(dp p) x -> p (dp x)", p=128)
    wout2 = w_out.rearrange("(dp p) x -> p (dp x)", p=128)
    bkT3 = bkT_out.rearrange("(dp p) t -> p dp t", p=128)

    with tile.TileContext(nc) as tc:
        with (
            tc.tile_pool(name="cst", bufs=1) as cst,
            tc.tile_pool(name="dram", bufs=1, space="DRAM") as dram,
            tc.tile_pool(name="persist", bufs=1) as persist,
        ):
            kT_ext = dram.tile([128, DP, WINDOW + S], BF16)
            xn_d = dram.tile([128, DP, S + 2], BF16)
            v_ext = dram.tile([WINDOW + S, D], BF16)
            qT_d = dram.tile([128, DP, S], BF16)
            bgT_d = dram.tile([128, DP, S], BF16)

            idf = cst.tile([128, 128], F32)
            make_identity(nc, idf[:])
            idb = cst.tile([128, 128], BF16)
            nc.vector.tensor_copy(idb[:], idf[:])
            aIb = cst.tile([128, 128], BF16)
            nc.vector.tensor_scalar_mul(aIb[:], idf[:], A_NS)
            aIf = cst.tile([128, 128], F32)
            nc.vector.tensor_scalar_mul(aIf[:], idf[:], A_NS)
            ones_col_b = cst.tile([128, 1], BF16)
            nc.gpsimd.memset(ones_col_b[:], 1.0)
            ones_row_b = cst.tile([1, 128], BF16)
            nc.gpsimd.memset(ones_row_b[:], 1.0)
            ones_row_f = cst.tile([1, 128], F32)
            nc.gpsimd.memset(ones_row_f[:], 1.0)
            win_sb = cst.tile([128, DP], F32)
            nc.sync.dma_start(win_sb[:], win2[:])
            wkq_sb = cst.tile([128, DP], F32)
            nc.sync.dma_start(wkq_sb[:], wkq2[:])
            wout_sb = cst.tile([128, DP], F32)
            nc.sync.dma_start(wout_sb[:], wout2[:])
            eps_col = cst.tile([128, 1], F32)
            nc.gpsimd.memset(eps_col[:], EPS)
            eps7_col = cst.tile([128, 1], F32)
            nc.gpsimd.memset(eps7_col[:], 1e-7)
            ckb_sb = cst.tile([128, DP], F32)
            nc.sync.dma_start(ckb_sb[:], ckb2[:])
            cqb_sb = cst.tile([128, DP], F32)
            nc.sync.dma_start(cqb_sb[:], cqb2[:])

            etaM = persist.tile([128, DP, NCH], F32)
            alM = persist.tile([128, DP, NCH], F32)

            nc.sync.dma_start(kT_ext[:, :, 0:WINDOW],
                              buf_kT.rearrange("(dp p) t -> p dp t", p=128))
            nc.sync.dma_start(v_ext[0:WINDOW, :], buf_v[:])

            # ================= PHASE 1 =================
            with (
                tc.tile_pool(name="p1", bufs=2) as p1,
                tc.tile_pool(name="p1x", bufs=1) as p1x,
                tc.tile_pool(name="p1w", bufs=2) as p1w,
                tc.tile_pool(name="p1ps", bufs=4, space="PSUM") as p1ps,
                tc.tile_pool(name="p1st", bufs=2, space="PSUM") as p1st,
            ):
                # normalized input with halo in DRAM: col c = token c-1
                bv_f32 = p1x.tile([128, D], F32)
                bk_f32 = p1x.tile([128, DP, WINDOW], F32)
                zc = p1x.tile([128, DP, 1], BF16)
                nc.gpsimd.memset(zc[:], 0.0)
                nc.sync.dma_start(xn_d[:, :, 0:1], zc[:])
                nc.sync.dma_start(xn_d[:, :, S + 1:S + 2], zc[:])

                def rstats_inv(src_tile, n):
                    sq = p1.tile([128, DP, n], BF16, tag="sq", name="sq")
                    nc.scalar.activation(sq[:], src_tile[:], AF.Square)
                    ss = p1st.tile([1, TBLK], F32, tag="ss", name="ss")[:, :n]
                    for dp in range(DP):
                        nc.tensor.matmul(ss, ones_col_b[:], sq[:, dp, :],
                                         start=(dp == 0), stop=(dp == DP - 1))
                    inv = p1.tile([1, n], F32, tag="inv", name="inv")
                    nc.scalar.activation(inv[:], ss, AF.Sqrt,
                                         bias=eps_col[:1, :], scale=1.0 / D)
                    nc.vector.reciprocal(inv[:], inv[:])
                    invb = p1.tile([1, n], BF16, tag="invb", name="invb")
                    nc.vector.tensor_copy(invb[:], inv[:])
                    ibc = p1st.tile([128, TBLK], F32, tag="ibc", name="ibc")[:, :n]
                    nc.tensor.matmul(ibc, ones_row_b[:], invb[:],
                                     start=True, stop=True)
                    return ibc  # [128, n] f32 psum, row-broadcast of inv_rms

                for blk in range(NBLK):
                    t0 = blk * TBLK
                    xb = p1.tile([128, DP, TBLK], BF16, tag="xb", name="xb")
                    nc.sync.dma_start(xb[:], xT3[:, :, t0:t0 + TBLK])
                    ibc = rstats_inv(xb, TBLK)
                    xnw = p1.tile([128, DP, TBLK], BF16, tag="xnw", name="xnw")
                    for dp in range(DP):
                        nc.vector.tensor_tensor(xnw[:, dp, :], xb[:, dp, :],
                                                ibc, OP.mult)
                        nc.vector.tensor_tensor(
                            xnw[:, dp, :], xnw[:, dp, :],
                            win_sb[:, dp, None].to_broadcast((128, TBLK)), OP.mult)
                    nc.sync.dma_start(xn_d[:, :, 1 + t0:1 + t0 + TBLK], xnw[:])

                for blk in range(NBLK):
                    t0 = blk * TBLK
                    xnb = p1.tile([128, DP, TBLK + 2], BF16, tag="xnb", name="xnb")
                    nc.sync.dma_start(xnb[:], xn_d[:, :, t0:t0 + TBLK + 2])
                    # ---- k / q with folded conv ----
                    for (wts, bias_sb, is_k) in (
                        (wk3, ckb_sb, True), (wq3, cqb_sb, False),
                    ):
                        kconv = p1.tile([128, DP, TBLK], BF16, tag="kconv", name="kconv")
                        for ot in range(DP):
                            ps = p1ps.tile([128, TBLK], F32, tag="mm", name="mm")
                            first = True
                            for r in range(3):
                                wt = p1w.tile([128, DP, 128], BF16, tag="wt", name="wt")
                                nc.sync.dma_start(
                                    wt[:], wts[r][:, :, ot * 128:(ot + 1) * 128])
                                for dp in range(DP):
                                    nc.tensor.matmul(
                                        ps[:], wt[:, dp, :],
                                        xnb[:, dp, r:r + TBLK],
                                        start=first,
                                        stop=(r == 2 and dp == DP - 1))
                                    first = False
                            nc.scalar.activation(kconv[:, ot, :], ps[:], AF.Silu,
                                                 bias=bias_sb[:, ot:ot + 1])
                        ibc = rstats_inv(kconv, TBLK)
                        kfin = p1.tile([128, DP, TBLK], BF16, tag="kfin", name="kfin")
                        for dp in range(DP):
                            nc.vector.tensor_tensor(kfin[:, dp, :],
                                                    kconv[:, dp, :], ibc, OP.mult)
                            nc.vector.tensor_tensor(
                                kfin[:, dp, :], kfin[:, dp, :],
                                wkq_sb[:, dp, None].to_broadcast((128, TBLK)),
                                OP.mult)
                        if is_k:
                            nc.sync.dma_start(
                                kT_ext[:, :, WINDOW + t0:WINDOW + t0 + TBLK],
                                kfin[:])
                            if blk == NBLK - 1:
                                for dp in range(DP):
                                    nc.vector.tensor_copy(
                                        bk_f32[:, dp, :],
                                        kfin[:, dp, TBLK - WINDOW:])
                        else:
                            nc.sync.dma_start(qT_d[:, :, t0:t0 + TBLK], kfin[:])

                    # ---- gates ----
                    gam = p1.tile([128, DP, TBLK], BF16, tag="gam", name="gam")
                    for gi in range(3):  # gamma, eta, alpha
                        for ot in range(DP):
                            ps = p1ps.tile([128, TBLK], F32, tag="mm", name="mm")
                            wt = p1w.tile([128, DP, 128], BF16, tag="wt", name="wt")
                            nc.sync.dma_start(
                                wt[:],
                                wg3[:, :, gi * D + ot * 128:gi * D + (ot + 1) * 128])
                            for dp in range(DP):
                                nc.tensor.matmul(
                                    ps[:], wt[:, dp, :],
                                    xnb[:, dp, 1:1 + TBLK],
                                    start=(dp == 0), stop=(dp == DP - 1))
                            if gi == 0:
                                nc.scalar.activation(gam[:, ot, :], ps[:], AF.Silu)
                            else:
                                sil = p1.tile([128, TBLK], F32, tag="sil", name="sil")
                                nc.scalar.activation(sil[:], ps[:], AF.Silu)
                                dst = etaM if gi == 1 else alM
                                nc.vector.tensor_reduce(
                                    dst[:, ot, blk * 8:(blk + 1) * 8],
                                    sil[:].rearrange("p (c x) -> p c x", x=CHUNK),
                                    mybir.AxisListType.X, OP.add)

                    # ---- bypass -> bg = silu(byp) * gamma * w_out ----
                    bg = p1.tile([128, DP, TBLK], BF16, tag="bg", name="bg")
                    for ot in range(DP):
                        ps = p1ps.tile([128, TBLK], F32, tag="mm", name="mm")
                        wt = p1w.tile([128, DP, 128], BF16, tag="wt", name="wt")
                        nc.sync.dma_start(wt[:], wb3[:, :, ot * 128:(ot + 1) * 128])
                        for dp in range(DP):
                            nc.tensor.matmul(ps[:], wt[:, dp, :],
                                             xnb[:, dp, 1:1 + TBLK],
                                             start=(dp == 0), stop=(dp == DP - 1))
                        sil = p1.tile([128, TBLK], F32, tag="sil", name="sil")
                        nc.scalar.activation(sil[:], ps[:], AF.Silu)
                        nc.vector.tensor_tensor(bg[:, ot, :], sil[:],
                                                gam[:, ot, :], OP.mult)
                        nc.vector.tensor_tensor(
                            bg[:, ot, :], bg[:, ot, :],
                            wout_sb[:, ot, None].to_broadcast((128, TBLK)),
                            OP.mult)
                    nc.sync.dma_start(bgT_d[:, :, t0:t0 + TBLK], bg[:])

                    # ---- v (natural layout) ----
                    vts = [p1.tile([128, D], BF16, tag=f"vt{tt}", name=f"vt{tt}")
                           for tt in range(TBLK // 128)]
                    for nj in range(2):
                        wvt = p1w.tile([128, DP, TBLK], BF16, tag="wvt", name="wvt")
                        nc.sync.dma_start(wvt[:],
                                          wv3[:, :, nj * TBLK:(nj + 1) * TBLK])
                        for tt in range(TBLK // 128):
                            ps = p1ps.tile([128, TBLK], F32, tag="mm", name="mm")
                            for dp in range(DP):
                                nc.tensor.matmul(
                                    ps[:],
                                    xnb[:, dp,
                                        1 + tt * 128:1 + (tt + 1) * 128],
                                    wvt[:, dp, :],
                                    start=(dp == 0), stop=(dp == DP - 1))
                            nc.scalar.activation(
                                vts[tt][:, nj * TBLK:(nj + 1) * TBLK], ps[:],
                                AF.Silu)
                    for tt in range(TBLK // 128):
                        row0 = WINDOW + t0 + tt * 128
                        nc.sync.dma_start(v_ext[row0:row0 + 128, :], vts[tt][:])
                    if blk == NBLK - 1:
                        nc.vector.tensor_copy(bv_f32[:], vts[TBLK // 128 - 1][:])

                nc.sync.dma_start(bkT3[:], bk_f32[:])
                nc.sync.dma_start(bv_out[:], bv_f32[:])

            # ================= PHASE 2 =================
            with (
                tc.tile_pool(name="p2", bufs=2) as p2,
                tc.tile_pool(name="p2m", bufs=1) as p2m,
                tc.tile_pool(name="mmps", bufs=3, space="PSUM") as mmps,
                tc.tile_pool(name="tpps", bufs=3, space="PSUM") as tpps,
                tc.tile_pool(name="aux", bufs=2, space="PSUM") as aux,
            ):
                M_sb = p2m.tile([128, DP, D], F32)
                M_bf = p2m.tile([128, DP, D], BF16)
                nc.sync.dma_start(M_sb[:], M03[:])
                for dp in range(DP):
                    nc.vector.tensor_copy(M_bf[:, dp, :], M_sb[:, dp, :])
                def transpose_to(dst_ap, src_ap, pw, fw, ident):
                    dt = BF16 if ident is idb else F32
                    ps = tpps.tile([128, 128], dt, tag="tp", name="tp")[:fw, :pw]
                    nc.tensor.transpose(ps, src_ap, ident[:pw, :pw])
                    nc.vector.tensor_copy(dst_ap, ps)

                _nch = int(os.environ.get("ATLAS_NCH", NCH))
                _nss = int(os.environ.get("ATLAS_NS", NS_STEPS))
                for ci in range(_nch):
                    c0 = ci * CHUNK
                    ctxT = p2.tile([128, DP, CTX], BF16, tag="ctxT", name="ctxT")
                    nc.sync.dma_start(ctxT[:], kT_ext[:, :, c0:c0 + CTX])
                    ctxv = [p2.tile([tw, D], BF16, tag=f"ctxv{ti}", name=f"ctxv{ti}")
                            for ti, tw in enumerate(TT)]
                    nc.sync.dma_start(ctxv[0][:], v_ext[c0:c0 + 128, :])
                    nc.sync.dma_start(ctxv[1][:], v_ext[c0 + 128:c0 + CTX, :])
                    qch = p2.tile([128, DP, CHUNK], BF16, tag="qch", name="qch")
                    nc.sync.dma_start(qch[:], qT_d[:, :, c0:c0 + CHUNK])
                    bgch = p2.tile([128, DP, CHUNK], BF16, tag="bgch", name="bgch")
                    nc.sync.dma_start(bgch[:], bgT_d[:, :, c0:c0 + CHUNK])

                    # err = ctx_k @ M - ctx_v
                    errt = [p2.tile([tw, D], BF16, tag=f"errt{ti}", name=f"errt{ti}")
                            for ti, tw in enumerate(TT)]
                    for ti, tw in enumerate(TT):
                        toff = ti * 128
                        for nj in range(2):
                            ps = mmps.tile([128, 512], F32, tag="mm", name="mm")[:tw]
                            for dp in range(DP):
                                nc.tensor.matmul(
                                    ps, ctxT[:, dp, toff:toff + tw],
                                    M_bf[:, dp, nj * 512:(nj + 1) * 512],
                                    start=(dp == 0), stop=(dp == DP - 1))
                            nc.vector.tensor_tensor(
                                errt[ti][:, nj * 512:(nj + 1) * 512], ps,
                                ctxv[ti][:, nj * 512:(nj + 1) * 512], OP.subtract)

                    # S = K K^T
                    Sg = [p2.tile([tw, CTX], BF16, tag=f"Sg{ti}", name=f"Sg{ti}")
                          for ti, tw in enumerate(TT)]
                    Sf = [p2.tile([tw, CTX], F32, tag=f"Sf{ti}", name=f"Sf{ti}")
                          for ti, tw in enumerate(TT)]
                    for ti, tw in enumerate(TT):
                        toff = ti * 128
                        ps = mmps.tile([128, 512], F32, tag="mm", name="mm")[:tw, :CTX]
                        for dp in range(DP):
                            nc.tensor.matmul(ps, ctxT[:, dp, toff:toff + tw],
                                             ctxT[:, dp, :],
                                             start=(dp == 0), stop=(dp == DP - 1))
                        nc.vector.tensor_copy(Sg[ti][:], ps)
                        nc.vector.tensor_copy(Sf[ti][:], ps)

                    # ET
                    ET = p2.tile([128, DP, CTX], BF16, tag="ET", name="ET")
                    for ti, tw in enumerate(TT):
                        for jt in range(DP):
                            transpose_to(ET[:, jt, ti * 128:ti * 128 + tw],
                                         errt[ti][:, jt * 128:(jt + 1) * 128],
                                         tw, 128, idb)

                    # ||G||^2 = sum(S * (E E^T))
                    zsum = p2.tile([128, 2], F32, tag="zsum", name="zsum")
                    nc.gpsimd.memset(zsum[:], 0.0)
                    for ti, tw in enumerate(TT):
                        toff = ti * 128
                        ps = mmps.tile([128, 512], F32, tag="mm", name="mm")[:tw, :CTX]
                        for jt in range(DP):
                            nc.tensor.matmul(ps, ET[:, jt, toff:toff + tw],
                                             ET[:, jt, :],
                                             start=(jt == 0), stop=(jt == DP - 1))
                        z = p2.tile([tw, CTX], F32, tag=f"z{ti}", name=f"z{ti}")
                        nc.vector.tensor_tensor(z[:], ps, Sf[ti][:], OP.mult)
                        nc.vector.tensor_reduce(zsum[:tw, ti:ti + 1], z[:],
                                                mybir.AxisListType.X, OP.add)
                    zsb = p2.tile([128, 2], BF16, tag="zsb", name="zsb")
                    nc.vector.tensor_copy(zsb[:], zsum[:])
                    g2ps = aux.tile([128, 512], F32, tag="aux", name="aux")[:1, :2]
                    nc.tensor.matmul(g2ps, ones_col_b[:], zsb[:],
                                     start=True, stop=True)
                    g2 = p2.tile([1, 1], F32, tag="g2", name="g2")
                    nc.vector.tensor_reduce(g2[:], g2ps, mybir.AxisListType.X,
                                            OP.add)
                    nc.scalar.activation(g2[:], g2[:], AF.Sqrt)
                    nc.scalar.add(g2[:], g2[:], eps7_col[:1, :])
                    nc.vector.reciprocal(g2[:], g2[:])
                    g2b = p2.tile([1, 1], BF16, tag="g2b", name="g2b")
                    nc.vector.tensor_copy(g2b[:], g2[:])
                    gcol = aux.tile([128, 512], F32, tag="aux", name="aux")[:, :1]
                    nc.tensor.matmul(gcol, ones_row_b[:], g2b[:],
                                     start=True, stop=True)

                    # W0 / WT0
                    Wc = [p2.tile([tw, D], F32, tag=f"W{ti}", name=f"W{ti}")
                          for ti, tw in enumerate(TT)]
                    for ti, tw in enumerate(TT):
                        nc.vector.tensor_tensor(
                            Wc[ti][:], errt[ti][:],
                            gcol[:tw].to_broadcast((tw, D)), OP.mult)
                    WT = p2.tile([128, DP, CTX], F32, tag="WT", name="WT")
                    for ti, tw in enumerate(TT):
                        for jt in range(DP):
                            transpose_to(WT[:, jt, ti * 128:ti * 128 + tw],
                                         Wc[ti][:, jt * 128:(jt + 1) * 128],
                                         tw, 128, idf)

                    # ---- NS steps ----
                    for step in range(_nss):
                        P_ = [p2.tile([tw, CTX], F32, tag=f"P{ti}", name=f"P{ti}")
                              for ti, tw in enumerate(TT)]
                        for ti, tw in enumerate(TT):
                            toff = ti * 128
                            ps = mmps.tile([128, 512], F32, tag="mm", name="mm")[:tw, :CTX]
                            for jt in range(DP):
                                nc.tensor.matmul(ps, WT[:, jt, toff:toff + tw],
                                                 WT[:, jt, :],
                                                 start=(jt == 0),
                                                 stop=(jt == DP - 1))
                            nc.vector.tensor_copy(P_[ti][:], ps)
                        M1b = [p2.tile([tw, CTX], F32, tag=f"M1b{ti}", name=f"M1b{ti}")
                               for ti, tw in enumerate(TT)]
                        for ti, tw in enumerate(TT):
                            toff = ti * 128
                            ps = mmps.tile([128, 512], F32, tag="mm", name="mm")[:tw, :CTX]
                            for ct in range(2):
                                nc.tensor.matmul(ps, P_[ct][:, toff:toff + tw],
                                                 Sf[ct][:],
                                                 start=(ct == 0), stop=(ct == 1))
                            nc.vector.tensor_scalar_mul(M1b[ti][:], ps, B_NS)
                        M1bT = [p2.tile([tw, CTX], F32, tag=f"M1bT{ti}", name=f"M1bT{ti}")
                                for ti, tw in enumerate(TT)]
                        for ti, tw in enumerate(TT):
                            for ct, cw in enumerate(TT):
                                transpose_to(
                                    M1bT[ti][:, ct * 128:ct * 128 + cw],
                                    M1b[ct][:, ti * 128:ti * 128 + tw],
                                    cw, tw, idf)
                        M4 = [p2.tile([tw, CTX], F32, tag=f"M4{ti}", name=f"M4{ti}")
                              for ti, tw in enumerate(TT)]
                        cb2 = C_NS / (B_NS * B_NS)
                        for ti, tw in enumerate(TT):
                            toff = ti * 128
                            ps = mmps.tile([128, 512], F32, tag="mm", name="mm")[:tw, :CTX]
                            for ct in range(2):
                                nc.tensor.matmul(ps, M1bT[ct][:, toff:toff + tw],
                                                 M1b[ct][:],
                                                 start=(ct == 0), stop=(ct == 1))
                            t4 = p2.tile([tw, CTX], F32, tag=f"t4{ti}", name=f"t4{ti}")
                            nc.vector.tensor_scalar_mul(t4[:], ps, cb2)
                            nc.vector.tensor_tensor(M4[ti][:], t4[:], M1b[ti][:],
                                                    OP.add)
                        M4T = [p2.tile([tw, CTX], F32, tag=f"M4T{ti}", name=f"M4T{ti}")
                               for ti, tw in enumerate(TT)]
                        for ti, tw in enumerate(TT):
                            for ct, cw in enumerate(TT):
                                transpose_to(
                                    M4T[ti][:, ct * 128:ct * 128 + cw],
                                    M4[ct][:, ti * 128:ti * 128 + tw],
                                    cw, tw, idf)
                        Wn = [p2.tile([tw, D], F32, tag=f"W{ti}", name=f"W{ti}")
                              for ti, tw in enumerate(TT)]
                        for ti, tw in enumerate(TT):
                            toff = ti * 128
                            for nj in range(2):
                                ps = mmps.tile([128, 512], F32, tag="mm", name="mm")[:tw]
                                nc.tensor.matmul(ps, M4T[0][:, toff:toff + tw],
                                                 Wc[0][:, nj * 512:(nj + 1) * 512],
                                                 start=True, stop=False)
                                nc.tensor.matmul(ps, M4T[1][:, toff:toff + tw],
                                                 Wc[1][:, nj * 512:(nj + 1) * 512],
                                                 start=False, stop=False)
                                nc.tensor.matmul(ps, aIf[:tw, :tw],
                                                 Wc[ti][:, nj * 512:(nj + 1) * 512],
                                                 start=False, stop=True)
                                nc.vector.tensor_copy(
                                    Wn[ti][:, nj * 512:(nj + 1) * 512], ps)
                        Wc = Wn
                        if step < _nss - 1:
                            WT = p2.tile([128, DP, CTX], F32, tag="WT", name="WT")
                            for ti, tw in enumerate(TT):
                                for jt in range(DP):
                                    transpose_to(
                                        WT[:, jt, ti * 128:ti * 128 + tw],
                                        Wc[ti][:, jt * 128:(jt + 1) * 128],
                                        tw, 128, idf)

                    # bf16 copy of W5 for og
                    W5b = [p2.tile([tw, D], BF16, tag=f"W5b{ti}", name=f"W5b{ti}")
                           for ti, tw in enumerate(TT)]
                    for ti, tw in enumerate(TT):
                        nc.vector.tensor_copy(W5b[ti][:], Wc[ti][:])
                    # ctx_k natural
                    ctxN = [p2.tile([tw, D], BF16, tag=f"ctxN{ti}", name=f"ctxN{ti}")
                            for ti, tw in enumerate(TT)]
                    for ti, tw in enumerate(TT):
                        for it in range(DP):
                            transpose_to(ctxN[ti][:, it * 128:(it + 1) * 128],
                                         ctxT[:, it, ti * 128:ti * 128 + tw],
                                         128, tw, idb)

                    # eta/alpha rows -> [1, D] via vec transposes, then bcast
                    ebc = p2.tile([128, D], F32, tag="ebc", name="ebc")
                    abc = p2.tile([128, D], F32, tag="abc", name="abc")
                    for (src, dst) in ((etaM, ebc), (alM, abc)):
                        row = p2.tile([1, D], F32, tag="row", name="row")
                        for dp in range(DP):
                            ps = tpps.tile([128, 128], F32, tag="tp", name="tp")[:1, :128]
                            nc.tensor.transpose(ps, src[:, dp, ci:ci + 1],
                                                idf[:])
                            nc.vector.tensor_scalar_mul(
                                row[:, dp * 128:(dp + 1) * 128], ps,
                                1.0 / CHUNK)
                        rowb = p2.tile([1, D], BF16, tag="rowb", name="rowb")
                        nc.vector.tensor_copy(rowb[:], row[:])
                        for nj in range(2):
                            ps = aux.tile([128, 512], F32, tag="aux", name="aux")
                            nc.tensor.matmul(
                                ps[:], ones_row_b[:],
                                rowb[:, nj * 512:(nj + 1) * 512],
                                start=True, stop=True)
                            nc.vector.tensor_copy(dst[:, nj * 512:(nj + 1) * 512],
                                                  ps[:])

                    # og = K^T W5 (pieces) and M update
                    for it in range(DP):
                        for nj in range(2):
                            sl = slice(nj * 512, (nj + 1) * 512)
                            ps = mmps.tile([128, 512], F32, tag="mm", name="mm")
                            nc.tensor.matmul(ps[:],
                                             ctxN[0][:, it * 128:(it + 1) * 128],
                                             W5b[0][:, sl], start=True, stop=False)
                            nc.tensor.matmul(ps[:],
                                             ctxN[1][:, it * 128:(it + 1) * 128],
                                             W5b[1][:, sl], start=False, stop=True)
                            t5 = p2.tile([128, 512], F32, tag="t5", name="t5")
                            nc.vector.tensor_tensor(t5[:], ps[:], ebc[:, sl],
                                                    OP.mult)
                            nc.vector.tensor_tensor(M_sb[:, it, sl],
                                                    M_sb[:, it, sl],
                                                    abc[:, sl], OP.mult)
                            nc.vector.tensor_tensor(M_sb[:, it, sl],
                                                    M_sb[:, it, sl], t5[:],
                                                    OP.subtract)
                        nc.vector.tensor_copy(M_bf[:, it, :], M_sb[:, it, :])

                    # c_out + fused epilogue
                    co = p2.tile([128, DP, CHUNK], BF16, tag="co", name="co")
                    for mj in range(DP):
                        ps = mmps.tile([128, 512], F32, tag="mm", name="mm")[:, :CHUNK]
                        for dp in range(DP):
                            nc.tensor.matmul(ps,
                                             M_bf[:, dp, mj * 128:(mj + 1) * 128],
                                             qch[:, dp, :],
                                             start=(dp == 0), stop=(dp == DP - 1))
                        nc.vector.tensor_copy(co[:, mj, :], ps)
                    csq = p2.tile([128, DP, CHUNK], BF16, tag="csq", name="csq")
                    nc.scalar.activation(csq[:], co[:], AF.Square)
                    ssps = aux.tile([128, 512], F32, tag="aux", name="aux")[:1, :CHUNK]
                    for dp in range(DP):
                        nc.tensor.matmul(ssps, ones_col_b[:], csq[:, dp, :],
                                         start=(dp == 0), stop=(dp == DP - 1))
                    oinv = p2.tile([1, CHUNK], F32, tag="oinv", name="oinv")
                    nc.scalar.activation(oinv[:], ssps, AF.Sqrt,
                                         bias=eps_col[:1, :], scale=1.0 / D)
                    nc.vector.reciprocal(oinv[:], oinv[:])
                    oinvb = p2.tile([1, CHUNK], BF16, tag="oinvb", name="oinvb")
                    nc.vector.tensor_copy(oinvb[:], oinv[:])
                    oibc = aux.tile([128, 512], F32, tag="aux", name="aux")[:, :CHUNK]
                    nc.tensor.matmul(oibc, ones_row_b[:], oinvb[:],
                                     start=True, stop=True)
                    oout = p2.tile([128, DP, CHUNK], F32, tag="oout", name="oout")
                    for dp in range(DP):
                        t6 = p2.tile([128, CHUNK], F32, tag="t6", name="t6")
                        nc.vector.tensor_tensor(t6[:], co[:, dp, :], oibc,
                                                OP.mult)
                        nc.vector.tensor_tensor(oout[:, dp, :], t6[:],
                                                bgch[:, dp, :], OP.mult)
                    nc.sync.dma_start(outT3[:, :, c0:c0 + CHUNK], oout[:])

                nc.sync.dma_start(Mo3[:], M_sb[:])
    nc.compile()
    return nc


def _get_nc():
    if "nc" not in _NC_CACHE:
        _NC_CACHE["nc"] = build()
    return _NC_CACHE["nc"]


def kernel(x, mem_state, buf_k, buf_v, norm_in_w, norm_kq_w, norm_out_w,
           Wk, Wq, Wv, Wg, Wb, conv_k_w, conv_k_b, conv_q_w, conv_q_b):
    x = np.asarray(x, np.float32)
    bf = lambda a: np.ascontiguousarray(np.asarray(a, np.float32)).astype(ml_dtypes.bfloat16)
    f32 = lambda a: np.ascontiguousarray(np.asarray(a, np.float32))

    ckw = np.asarray(conv_k_w, np.float32)
    cqw = np.asarray(conv_q_w, np.float32)
    WkTs = [bf(np.asarray(Wk).T * ckw[:, 0, r][None, :]) for r in range(3)]
    WqTs = [bf(np.asarray(Wq).T * cqw[:, 0, r][None, :]) for r in range(3)]
    shared = {
        "WkT0": WkTs[0], "WkT1": WkTs[1], "WkT2": WkTs[2],
        "WqT0": WqTs[0], "WqT1": WqTs[1], "WqT2": WqTs[2],
        "WvT": bf(np.asarray(Wv).T), "WgT": bf(np.asarray(Wg).T),
        "WbT": bf(np.asarray(Wb).T),
        "conv_k_b": f32(conv_k_b)[:, None], "conv_q_b": f32(conv_q_b)[:, None],
        "norm_in_w": f32(norm_in_w)[:, None],
        "norm_kq_w": f32(norm_kq_w)[:, None],
        "norm_out_w": f32(norm_out_w)[:, None],
    }
    in_maps = []
    for c in range(8):
        b = c % B
        m = dict(shared)
        m["xT"] = bf(np.ascontiguousarray(x[b].T))
        m["M0"] = f32(mem_state[b])
        m["buf_kT"] = bf(np.asarray(buf_k)[b].T)
        m["buf_v"] = bf(np.asarray(buf_v)[b])
        in_maps.append(m)

    nc = _get_nc()
    if os.environ.get("ATLAS_TRACE"):
        try:
            r = run_bass_kernel_spmd(nc, in_maps, list(range(8)), trace=True)
            globals()["LAST_EXEC_NS"] = r.exec_time_ns
            res = r.results
        except (ImportError, ModuleNotFoundError):
            import time as _t
            res = run_bass_kernel_spmd(nc, in_maps, list(range(8))).results
            t0 = _t.time()
            res = run_bass_kernel_spmd(nc, in_maps, list(range(8))).results
            globals()["LAST_EXEC_NS"] = int((_t.time() - t0) * 1e9)
    else:
        res = run_bass_kernel_spmd(nc, in_maps, list(range(8))).results

    out = np.stack([res[b]["outT"].T for b in range(B)])
    M = np.stack([res[b]["M_out"] for b in range(B)])
    bk = np.stack([res[b]["bkT"].T for b in range(B)])
    bv = np.stack([res[b]["bv"] for b in range(B)])
    return (out.astype(np.float32), M.astype(np.float32),
            bk.astype(np.float32), bv.astype(np.float32))


# revision 17
# speedup vs baseline: 1.1710x; 1.0105x over previous
"""Trainium2 Bass kernel for nn_AtlasLayerBlock_65395172049082.

- Data-parallel over batch: core c computes batch c % 4 fully (2x redundant).
- Sequence tensors in transposed [d, t] layout on device; host transposes.
- dwconv folded into 3 time-shifted projections (host-prescaled weights).
- Newton-Schulz in rank-192 factor space: X_t = K^T W_t,
  W <- a W + (b M1 + c M1^2) W,  M1 = (W W^T)(K K^T).
- bf16 matmul operands, fp32 accumulation and fp32 memory state M.
- Fully static program (no hardware loops).
"""
import os
import numpy as np
import ml_dtypes

import concourse.bass as bass
import concourse.mybir as mybir
import concourse.tile as tile
from concourse.bass_utils import run_bass_kernel_spmd
from concourse.masks import make_identity

F32 = mybir.dt.float32
BF16 = mybir.dt.bfloat16
AF = mybir.ActivationFunctionType
OP = mybir.AluOpType

B, S, D = 4, 4096, 1024
CHUNK, WINDOW, NS_STEPS, EPS = 64, 128, 5, 1e-6
A_NS, B_NS, C_NS = 3.4445, -4.7750, 2.0315
NCH = S // CHUNK
CTX = WINDOW + CHUNK        # 192
TBLK = 512
NBLK = S // TBLK
DP = D // 128
TT = (128, 64)              # 192 split into t-subtiles

_NC_CACHE = {}


def build():
    nc = bass.Bass("TRN2", num_devices=8)
    dd = nc.declare_dram_parameter
    xT = dd("xT", [D, S], BF16, isOutput=False)
    M0 = dd("M0", [D, D], F32, isOutput=False)
    buf_kT = dd("buf_kT", [D, WINDOW], BF16, isOutput=False)
    buf_v = dd("buf_v", [WINDOW, D], BF16, isOutput=False)
    wk = [dd(f"WkT{r}", [D, D], BF16, isOutput=False) for r in range(3)]
    wq = [dd(f"WqT{r}", [D, D], BF16, isOutput=False) for r in range(3)]
    wv = dd("WvT", [D, D], BF16, isOutput=False)
    wg = dd("WgT", [D, 3 * D], BF16, isOutput=False)
    wb = dd("WbT", [D, D], BF16, isOutput=False)
    conv_k_b = dd("conv_k_b", [D, 1], F32, isOutput=False)
    conv_q_b = dd("conv_q_b", [D, 1], F32, isOutput=False)
    w_in = dd("norm_in_w", [D, 1], F32, isOutput=False)
    w_kq = dd("norm_kq_w", [D, 1], F32, isOutput=False)
    w_out = dd("norm_out_w", [D, 1], F32, isOutput=False)
    outT = dd("outT", [D, S], F32, isOutput=True)
    M_out = dd("M_out", [D, D], F32, isOutput=True)
    bkT_out = dd("bkT", [D, WINDOW], F32, isOutput=True)
    bv_out = dd("bv", [WINDOW, D], F32, isOutput=True)

    xT3 = xT.rearrange("(dp p) t -> p dp t", p=128)
    outT3 = outT.rearrange("(dp p) t -> p dp t", p=128)
    M03 = M0.rearrange("(dp p) j -> p dp j", p=128)
    Mo3 = M_out.rearrange("(dp p) j -> p dp j", p=128)
    wv3 = wv.rearrange("(dp p) o -> p dp o", p=128)
    wg3 = wg.rearrange("(dp p) o -> p dp o", p=128)
    wb3 = wb.rearrange("(dp p) o -> p dp o", p=128)
    wk3 = [w.rearrange("(dp p) o -> p dp o", p=128) for w in wk]
    wq3 = [w.rearrange("(dp p) o -> p dp o", p=128) for w in wq]
    ckb2 = conv_k_b.rearrange("(dp p) x -> p (dp x)", p=128)
    cqb2 = conv_q_b.rearrange("(dp p) x -> p (dp x)", p=128)
    win2 = w_in.rearrange("(dp p) x -> p (dp x)", p=128)
    wkq2 = w_kq.rearrange("(dp p) x -> p (dp x)", p=128)
    wout2 = w_out.rearrange("(dp p) x -> p (dp x)", p=128)
    bkT3 = bkT_out.rearrange("(dp p) t -> p dp t", p=128)

    with tile.TileContext(nc) as tc:
        with (
            tc.tile_pool(name="cst", bufs=1) as cst,
            tc.tile_pool(name="dram", bufs=1, space="DRAM") as dram,
            tc.tile_pool(name="persist", bufs=1) as persist,
        ):
            kT_ext = dram.tile([128, DP, WINDOW + S], BF16)
            xn_d = dram.tile([128, DP, S + 2], BF16)
            v_ext = dram.tile([WINDOW + S, D], BF16)
            qT_d = dram.tile([128, DP, S], BF16)
            bgT_d = dram.tile([128, DP, S], BF16)

            idf = cst.tile([128, 128], F32)
            make_identity(nc, idf[:])
            idb = cst.tile([128, 128], BF16)
            nc.vector.tensor_copy(idb[:], idf[:])
            aIb = cst.tile([128, 128], BF16)
            nc.vector.tensor_scalar_mul(aIb[:], idf[:], A_NS)
            aIf = cst.tile([128, 128], F32)
            nc.vector.tensor_scalar_mul(aIf[:], idf[:], A_NS)
            ones_col_b = cst.tile([128, 1], BF16)
            nc.gpsimd.memset(ones_col_b[:], 1.0)
            ones_row_b = cst.tile([1, 128], BF16)
            nc.gpsimd.memset(ones_row_b[:], 1.0)
            ones_row_f = cst.tile([1, 128], F32)
            nc.gpsimd.memset(ones_row_f[:], 1.0)
            win_sb = cst.tile([128, DP], F32)
            nc.sync.dma_start(win_sb[:], win2[:])
            wkq_sb = cst.tile([128, DP], F32)
            nc.sync.dma_start(wkq_sb[:], wkq2[:])
            wout_sb = cst.tile([128, DP], F32)
            nc.sync.dma_start(wout_sb[:], wout2[:])
            eps_col = cst.tile([128, 1], F32)
            nc.gpsimd.memset(eps_col[:], EPS)
            eps7_col = cst.tile([128, 1], F32)
            nc.gpsimd.memset(eps7_col[:], 1e-7)
            ckb_sb = cst.tile([128, DP], F32)
            nc.sync.dma_start(ckb_sb[:], ckb2[:])
            cqb_sb = cst.tile([128, DP], F32)
            nc.sync.dma_start(cqb_sb[:], cqb2[:])

            etaM = persist.tile([128, DP, NCH], F32)
            alM = persist.tile([128, DP, NCH], F32)

            nc.sync.dma_start(kT_ext[:, :, 0:WINDOW],
                              buf_kT.rearrange("(dp p) t -> p dp t", p=128))
            nc.sync.dma_start(v_ext[0:WINDOW, :], buf_v[:])

            # ================= PHASE 1 =================
            with (
                tc.tile_pool(name="p1", bufs=2) as p1,
                tc.tile_pool(name="p1x", bufs=1) as p1x,
                tc.tile_pool(name="p1w", bufs=2) as p1w,
                tc.tile_pool(name="p1ps", bufs=4, space="PSUM") as p1ps,
                tc.tile_pool(name="p1st", bufs=2, space="PSUM") as p1st,
            ):
                # normalized input with halo in DRAM: col c = token c-1
                bv_f32 = p1x.tile([128, D], F32)
                bk_f32 = p1x.tile([128, DP, WINDOW], F32)
                zc = p1x.tile([128, DP, 1], BF16)
                nc.gpsimd.memset(zc[:], 0.0)
                nc.sync.dma_start(xn_d[:, :, 0:1], zc[:])
                nc.sync.dma_start(xn_d[:, :, S + 1:S + 2], zc[:])

                def rstats_inv(src_tile, n):
                    sq = p1.tile([128, DP, n], BF16, tag="sq", name="sq")
                    nc.scalar.activation(sq[:], src_tile[:], AF.Square)
                    ss = p1st.tile([1, TBLK], F32, tag="ss", name="ss")[:, :n]
                    for dp in range(DP):
                        nc.tensor.matmul(ss, ones_col_b[:], sq[:, dp, :],
                                         start=(dp == 0), stop=(dp == DP - 1))
                    inv = p1.tile([1, n], F32, tag="inv", name="inv")
                    nc.scalar.activation(inv[:], ss, AF.Sqrt,
                                         bias=eps_col[:1, :], scale=1.0 / D)
                    nc.vector.reciprocal(inv[:], inv[:])
                    invb = p1.tile([1, n], BF16, tag="invb", name="invb")
                    nc.vector.tensor_copy(invb[:], inv[:])
                    ibc = p1st.tile([128, TBLK], F32, tag="ibc", name="ibc")[:, :n]
                    nc.tensor.matmul(ibc, ones_row_b[:], invb[:],
                                     start=True, stop=True)
                    return ibc  # [128, n] f32 psum, row-broadcast of inv_rms

                for blk in range(NBLK):
                    t0 = blk * TBLK
                    xb = p1.tile([128, DP, TBLK], BF16, tag="xb", name="xb")
                    nc.sync.dma_start(xb[:], xT3[:, :, t0:t0 + TBLK])
                    ibc = rstats_inv(xb, TBLK)
                    xnw = p1.tile([128, DP, TBLK], BF16, tag="xnw", name="xnw")
                    for dp in range(DP):
                        nc.vector.tensor_tensor(xnw[:, dp, :], xb[:, dp, :],
                                                ibc, OP.mult)
                        nc.vector.tensor_tensor(
                            xnw[:, dp, :], xnw[:, dp, :],
                            win_sb[:, dp, None].to_broadcast((128, TBLK)), OP.mult)
                    nc.sync.dma_start(xn_d[:, :, 1 + t0:1 + t0 + TBLK], xnw[:])

                _p1b = int(os.environ.get("ATLAS_P1BLOCKS", NBLK))
                for blk in range(_p1b):
                    t0 = blk * TBLK
                    xnb = p1.tile([128, DP, TBLK + 2], BF16, tag="xnb", name="xnb")
                    nc.sync.dma_start(xnb[:], xn_d[:, :, t0:t0 + TBLK + 2])
                    # ---- k / q with folded conv ----
                    for (wts, bias_sb, is_k) in (
                        (wk3, ckb_sb, True), (wq3, cqb_sb, False),
                    ):
                        kconv = p1.tile([128, DP, TBLK], BF16, tag="kconv", name="kconv")
                        for ot in range(DP):
                            ps = p1ps.tile([128, TBLK], F32, tag="mm", name="mm")
                            first = True
                            for r in range(3):
                                wt = p1w.tile([128, DP, 128], BF16, tag="wt", name="wt")
                                nc.sync.dma_start(
                                    wt[:], wts[r][:, :, ot * 128:(ot + 1) * 128])
                                for dp in range(DP):
                                    nc.tensor.matmul(
                                        ps[:], wt[:, dp, :],
                                        xnb[:, dp, r:r + TBLK],
                                        start=first,
                                        stop=(r == 2 and dp == DP - 1))
                                    first = False
                            nc.scalar.activation(kconv[:, ot, :], ps[:], AF.Silu,
                                                 bias=bias_sb[:, ot:ot + 1])
                        ibc = rstats_inv(kconv, TBLK)
                        kfin = p1.tile([128, DP, TBLK], BF16, tag="kfin", name="kfin")
                        for dp in range(DP):
                            nc.vector.tensor_tensor(kfin[:, dp, :],
                                                    kconv[:, dp, :], ibc, OP.mult)
                            nc.vector.tensor_tensor(
                                kfin[:, dp, :], kfin[:, dp, :],
                                wkq_sb[:, dp, None].to_broadcast((128, TBLK)),
                                OP.mult)
                        if is_k:
                            nc.sync.dma_start(
                                kT_ext[:, :, WINDOW + t0:WINDOW + t0 + TBLK],
                                kfin[:])
                            if blk == NBLK - 1:
                                for dp in range(DP):
                                    nc.vector.tensor_copy(
                                        bk_f32[:, dp, :],
                                        kfin[:, dp, TBLK - WINDOW:])
                        else:
                            nc.sync.dma_start(qT_d[:, :, t0:t0 + TBLK], kfin[:])

                    # ---- gates ----
                    gam = p1.tile([128, DP, TBLK], BF16, tag="gam", name="gam")
                    for gi in range(3):  # gamma, eta, alpha
                        for ot in range(DP):
                            ps = p1ps.tile([128, TBLK], F32, tag="mm", name="mm")
                            wt = p1w.tile([128, DP, 128], BF16, tag="wt", name="wt")
                            nc.sync.dma_start(
                                wt[:],
                                wg3[:, :, gi * D + ot * 128:gi * D + (ot + 1) * 128])
                            for dp in range(DP):
                                nc.tensor.matmul(
                                    ps[:], wt[:, dp, :],
                                    xnb[:, dp, 1:1 + TBLK],
                                    start=(dp == 0), stop=(dp == DP - 1))
                            if gi == 0:
                                nc.scalar.activation(gam[:, ot, :], ps[:], AF.Silu)
                            else:
                                sil = p1.tile([128, TBLK], F32, tag="sil", name="sil")
                                nc.scalar.activation(sil[:], ps[:], AF.Silu)
                                dst = etaM if gi == 1 else alM
                                nc.vector.tensor_reduce(
                                    dst[:, ot, blk * 8:(blk + 1) * 8],
                                    sil[:].rearrange("p (c x) -> p c x", x=CHUNK),
                                    mybir.AxisListType.X, OP.add)

                    # ---- bypass -> bg = silu(byp) * gamma * w_out ----
                    bg = p1.tile([128, DP, TBLK], BF16, tag="bg", name="bg")
                    for ot in range(DP):
                        ps = p1ps.tile([128, TBLK], F32, tag="mm", name="mm")
                        wt = p1w.tile([128, DP, 128], BF16, tag="wt", name="wt")
                        nc.sync.dma_start(wt[:], wb3[:, :, ot * 128:(ot + 1) * 128])
                        for dp in range(DP):
                            nc.tensor.matmul(ps[:], wt[:, dp, :],
                                             xnb[:, dp, 1:1 + TBLK],
                                             start=(dp == 0), stop=(dp == DP - 1))
                        sil = p1.tile([128, TBLK], F32, tag="sil", name="sil")
                        nc.scalar.activation(sil[:], ps[:], AF.Silu)
                        nc.vector.tensor_tensor(bg[:, ot, :], sil[:],
                                                gam[:, ot, :], OP.mult)
                        nc.vector.tensor_tensor(
                            bg[:, ot, :], bg[:, ot, :],
                            wout_sb[:, ot, None].to_broadcast((128, TBLK)),
                            OP.mult)
                    nc.sync.dma_start(bgT_d[:, :, t0:t0 + TBLK], bg[:])

                    # ---- v (natural layout) ----
                    vts = [p1.tile([128, D], BF16, tag=f"vt{tt}", name=f"vt{tt}")
                           for tt in range(TBLK // 128)]
                    for nj in range(2):
                        wvt = p1w.tile([128, DP, TBLK], BF16, tag="wvt", name="wvt")
                        nc.sync.dma_start(wvt[:],
                                          wv3[:, :, nj * TBLK:(nj + 1) * TBLK])
                        for tt in range(TBLK // 128):
                            ps = p1ps.tile([128, TBLK], F32, tag="mm", name="mm")
                            for dp in range(DP):
                                nc.tensor.matmul(
                                    ps[:],
                                    xnb[:, dp,
                                        1 + tt * 128:1 + (tt + 1) * 128],
                                    wvt[:, dp, :],
                                    start=(dp == 0), stop=(dp == DP - 1))
                            nc.scalar.activation(
                                vts[tt][:, nj * TBLK:(nj + 1) * TBLK], ps[:],
                                AF.Silu)
                    for tt in range(TBLK // 128):
                        row0 = WINDOW + t0 + tt * 128
                        nc.sync.dma_start(v_ext[row0:row0 + 128, :], vts[tt][:])
                    if blk == NBLK - 1:
                        nc.vector.tensor_copy(bv_f32[:], vts[TBLK // 128 - 1][:])

                nc.sync.dma_start(bkT3[:], bk_f32[:])
                nc.sync.dma_start(bv_out[:], bv_f32[:])

            # ================= PHASE 2 =================
            with (
                tc.tile_pool(name="p2", bufs=2) as p2,
                tc.tile_pool(name="p2m", bufs=1) as p2m,
                tc.tile_pool(name="mmps", bufs=3, space="PSUM") as mmps,
                tc.tile_pool(name="tpps", bufs=3, space="PSUM") as tpps,
                tc.tile_pool(name="aux", bufs=2, space="PSUM") as aux,
            ):
                M_sb = p2m.tile([128, DP, D], F32)
                M_bf = p2m.tile([128, DP, D], BF16)
                nc.sync.dma_start(M_sb[:], M03[:])
                for dp in range(DP):
                    nc.vector.tensor_copy(M_bf[:, dp, :], M_sb[:, dp, :])
                def transpose_to(dst_ap, src_ap, pw, fw, ident):
                    dt = BF16 if ident is idb else F32
                    ps = tpps.tile([128, 128], dt, tag="tp", name="tp")[:fw, :pw]
                    nc.tensor.transpose(ps, src_ap, ident[:pw, :pw])
                    nc.vector.tensor_copy(dst_ap, ps)

                _nch = int(os.environ.get("ATLAS_NCH", NCH))
                _nss = int(os.environ.get("ATLAS_NS", NS_STEPS))
                for ci in range(_nch):
                    c0 = ci * CHUNK
                    ctxT = p2.tile([128, DP, CTX], BF16, tag="ctxT", name="ctxT")
                    nc.sync.dma_start(ctxT[:], kT_ext[:, :, c0:c0 + CTX])
                    ctxv = [p2.tile([tw, D], BF16, tag=f"ctxv{ti}", name=f"ctxv{ti}")
                            for ti, tw in enumerate(TT)]
                    nc.sync.dma_start(ctxv[0][:], v_ext[c0:c0 + 128, :])
                    nc.sync.dma_start(ctxv[1][:], v_ext[c0 + 128:c0 + CTX, :])
                    qch = p2.tile([128, DP, CHUNK], BF16, tag="qch", name="qch")
                    nc.sync.dma_start(qch[:], qT_d[:, :, c0:c0 + CHUNK])
                    bgch = p2.tile([128, DP, CHUNK], BF16, tag="bgch", name="bgch")
                    nc.sync.dma_start(bgch[:], bgT_d[:, :, c0:c0 + CHUNK])

                    # err = ctx_k @ M - ctx_v
                    errt = [p2.tile([tw, D], BF16, tag=f"errt{ti}", name=f"errt{ti}")
                            for ti, tw in enumerate(TT)]
                    for ti, tw in enumerate(TT):
                        toff = ti * 128
                        for nj in range(2):
                            ps = mmps.tile([128, 512], F32, tag="mm", name="mm")[:tw]
                            for dp in range(DP):
                                nc.tensor.matmul(
                                    ps, ctxT[:, dp, toff:toff + tw],
                                    M_bf[:, dp, nj * 512:(nj + 1) * 512],
                                    start=(dp == 0), stop=(dp == DP - 1))
                            nc.vector.tensor_tensor(
                                errt[ti][:, nj * 512:(nj + 1) * 512], ps,
                                ctxv[ti][:, nj * 512:(nj + 1) * 512], OP.subtract)

                    # S = K K^T
                    Sg = [p2.tile([tw, CTX], BF16, tag=f"Sg{ti}", name=f"Sg{ti}")
                          for ti, tw in enumerate(TT)]
                    Sf = [p2.tile([tw, CTX], F32, tag=f"Sf{ti}", name=f"Sf{ti}")
                          for ti, tw in enumerate(TT)]
                    for ti, tw in enumerate(TT):
                        toff = ti * 128
                        ps = mmps.tile([128, 512], F32, tag="mm", name="mm")[:tw, :CTX]
                        for dp in range(DP):
                            nc.tensor.matmul(ps, ctxT[:, dp, toff:toff + tw],
                                             ctxT[:, dp, :],
                                             start=(dp == 0), stop=(dp == DP - 1))
                        nc.vector.tensor_copy(Sg[ti][:], ps)
                        nc.vector.tensor_copy(Sf[ti][:], ps)

                    # ET
                    ET = p2.tile([128, DP, CTX], BF16, tag="ET", name="ET")
                    for ti, tw in enumerate(TT):
                        for jt in range(DP):
                            transpose_to(ET[:, jt, ti * 128:ti * 128 + tw],
                                         errt[ti][:, jt * 128:(jt + 1) * 128],
                                         tw, 128, idb)

                    # ||G||^2 = sum(S * (E E^T))
                    zsum = p2.tile([128, 2], F32, tag="zsum", name="zsum")
                    nc.gpsimd.memset(zsum[:], 0.0)
                    for ti, tw in enumerate(TT):
                        toff = ti * 128
                        ps = mmps.tile([128, 512], F32, tag="mm", name="mm")[:tw, :CTX]
                        for jt in range(DP):
                            nc.tensor.matmul(ps, ET[:, jt, toff:toff + tw],
                                             ET[:, jt, :],
                                             start=(jt == 0), stop=(jt == DP - 1))
                        z = p2.tile([tw, CTX], F32, tag=f"z{ti}", name=f"z{ti}")
                        nc.vector.tensor_tensor(z[:], ps, Sf[ti][:], OP.mult)
                        nc.vector.tensor_reduce(zsum[:tw, ti:ti + 1], z[:],
                                                mybir.AxisListType.X, OP.add)
                    zsb = p2.tile([128, 2], BF16, tag="zsb", name="zsb")
                    nc.vector.tensor_copy(zsb[:], zsum[:])
                    g2ps = aux.tile([128, 512], F32, tag="aux", name="aux")[:1, :2]
                    nc.tensor.matmul(g2ps, ones_col_b[:], zsb[:],
                                     start=True, stop=True)
                    g2 = p2.tile([1, 1], F32, tag="g2", name="g2")
                    nc.vector.tensor_reduce(g2[:], g2ps, mybir.AxisListType.X,
                                            OP.add)
                    nc.scalar.activation(g2[:], g2[:], AF.Sqrt)
                    nc.scalar.add(g2[:], g2[:], eps7_col[:1, :])
                    nc.vector.reciprocal(g2[:], g2[:])
                    g2b = p2.tile([1, 1], BF16, tag="g2b", name="g2b")
                    nc.vector.tensor_copy(g2b[:], g2[:])
                    gcol = aux.tile([128, 512], F32, tag="aux", name="aux")[:, :1]
                    nc.tensor.matmul(gcol, ones_row_b[:], g2b[:],
                                     start=True, stop=True)

                    # W0 / WT0
                    Wc = [p2.tile([tw, D], F32, tag=f"W{ti}", name=f"W{ti}")
                          for ti, tw in enumerate(TT)]
                    for ti, tw in enumerate(TT):
                        nc.vector.tensor_tensor(
                            Wc[ti][:], errt[ti][:],
                            gcol[:tw].to_broadcast((tw, D)), OP.mult)
                    WT = p2.tile([128, DP, CTX], F32, tag="WT", name="WT")
                    for ti, tw in enumerate(TT):
                        for jt in range(DP):
                            transpose_to(WT[:, jt, ti * 128:ti * 128 + tw],
                                         Wc[ti][:, jt * 128:(jt + 1) * 128],
                                         tw, 128, idf)

                    # ---- NS steps ----
                    for step in range(_nss):
                        P_ = [p2.tile([tw, CTX], F32, tag=f"P{ti}", name=f"P{ti}")
                              for ti, tw in enumerate(TT)]
                        for ti, tw in enumerate(TT):
                            toff = ti * 128
                            ps = mmps.tile([128, 512], F32, tag="mm", name="mm")[:tw, :CTX]
                            for jt in range(DP):
                                nc.tensor.matmul(ps, WT[:, jt, toff:toff + tw],
                                                 WT[:, jt, :],
                                                 start=(jt == 0),
                                                 stop=(jt == DP - 1))
                            nc.vector.tensor_copy(P_[ti][:], ps)
                        M1b = [p2.tile([tw, CTX], F32, tag=f"M1b{ti}", name=f"M1b{ti}")
                               for ti, tw in enumerate(TT)]
                        for ti, tw in enumerate(TT):
                            toff = ti * 128
                            ps = mmps.tile([128, 512], F32, tag="mm", name="mm")[:tw, :CTX]
                            for ct in range(2):
                                nc.tensor.matmul(ps, P_[ct][:, toff:toff + tw],
                                                 Sf[ct][:],
                                                 start=(ct == 0), stop=(ct == 1))
                            nc.vector.tensor_scalar_mul(M1b[ti][:], ps, B_NS)
                        M1bT = [p2.tile([tw, CTX], F32, tag=f"M1bT{ti}", name=f"M1bT{ti}")
                                for ti, tw in enumerate(TT)]
                        for ti, tw in enumerate(TT):
                            for ct, cw in enumerate(TT):
                                transpose_to(
                                    M1bT[ti][:, ct * 128:ct * 128 + cw],
                                    M1b[ct][:, ti * 128:ti * 128 + tw],
                                    cw, tw, idf)
                        M4 = [p2.tile([tw, CTX], F32, tag=f"M4{ti}", name=f"M4{ti}")
                              for ti, tw in enumerate(TT)]
                        cb2 = C_NS / (B_NS * B_NS)
                        for ti, tw in enumerate(TT):
                            toff = ti * 128
                            ps = mmps.tile([128, 512], F32, tag="mm", name="mm")[:tw, :CTX]
                            for ct in range(2):
                                nc.tensor.matmul(ps, M1bT[ct][:, toff:toff + tw],
                                                 M1b[ct][:],
                                                 start=(ct == 0), stop=(ct == 1))
                            t4 = p2.tile([tw, CTX], F32, tag=f"t4{ti}", name=f"t4{ti}")
                            nc.vector.tensor_scalar_mul(t4[:], ps, cb2)
                            nc.vector.tensor_tensor(M4[ti][:], t4[:], M1b[ti][:],
                                                    OP.add)
                        M4T = [p2.tile([tw, CTX], F32, tag=f"M4T{ti}", name=f"M4T{ti}")
                               for ti, tw in enumerate(TT)]
                        for ti, tw in enumerate(TT):
                            for ct, cw in enumerate(TT):
                                transpose_to(
                                    M4T[ti][:, ct * 128:ct * 128 + cw],
                                    M4[ct][:, ti * 128:ti * 128 + tw],
                                    cw, tw, idf)
                        Wn = [p2.tile([tw, D], F32, tag=f"W{ti}", name=f"W{ti}")
                              for ti, tw in enumerate(TT)]
                        for ti, tw in enumerate(TT):
                            toff = ti * 128
                            for nj in range(2):
                                ps = mmps.tile([128, 512], F32, tag="mm", name="mm")[:tw]
                                nc.tensor.matmul(ps, M4T[0][:, toff:toff + tw],
                                                 Wc[0][:, nj * 512:(nj + 1) * 512],
                                                 start=True, stop=False)
                                nc.tensor.matmul(ps, M4T[1][:, toff:toff + tw],
                                                 Wc[1][:, nj * 512:(nj + 1) * 512],
                                                 start=False, stop=False)
                                nc.tensor.matmul(ps, aIf[:tw, :tw],
                                                 Wc[ti][:, nj * 512:(nj + 1) * 512],
                                                 start=False, stop=True)
                                nc.vector.tensor_copy(
                                    Wn[ti][:, nj * 512:(nj + 1) * 512], ps)
                        Wc = Wn
                        if step < _nss - 1:
                            WT = p2.tile([128, DP, CTX], F32, tag="WT", name="WT")
                            for ti, tw in enumerate(TT):
                                for jt in range(DP):
                                    transpose_to(
                                        WT[:, jt, ti * 128:ti * 128 + tw],
                                        Wc[ti][:, jt * 128:(jt + 1) * 128],
                                        tw, 128, idf)

                    # bf16 copy of W5 for og
                    W5b = [p2.tile([tw, D], BF16, tag=f"W5b{ti}", name=f"W5b{ti}")
                           for ti, tw in enumerate(TT)]
                    for ti, tw in enumerate(TT):
                        nc.vector.tensor_copy(W5b[ti][:], Wc[ti][:])
                    # ctx_k natural
                    ctxN = [p2.tile([tw, D], BF16, tag=f"ctxN{ti}", name=f"ctxN{ti}")
                            for ti, tw in enumerate(TT)]
                    for ti, tw in enumerate(TT):
                        for it in range(DP):
                            transpose_to(ctxN[ti][:, it * 128:(it + 1) * 128],
                                         ctxT[:, it, ti * 128:ti * 128 + tw],
                                         128, tw, idb)

                    # eta/alpha rows -> [1, D] via vec transposes, then bcast
                    ebc = p2.tile([128, D], F32, tag="ebc", name="ebc")
                    abc = p2.tile([128, D], F32, tag="abc", name="abc")
                    for (src, dst) in ((etaM, ebc), (alM, abc)):
                        row = p2.tile([1, D], F32, tag="row", name="row")
                        for dp in range(DP):
                            ps = tpps.tile([128, 128], F32, tag="tp", name="tp")[:1, :128]
                            nc.tensor.transpose(ps, src[:, dp, ci:ci + 1],
                                                idf[:])
                            nc.vector.tensor_scalar_mul(
                                row[:, dp * 128:(dp + 1) * 128], ps,
                                1.0 / CHUNK)
                        rowb = p2.tile([1, D], BF16, tag="rowb", name="rowb")
                        nc.vector.tensor_copy(rowb[:], row[:])
                        for nj in range(2):
                            ps = aux.tile([128, 512], F32, tag="aux", name="aux")
                            nc.tensor.matmul(
                                ps[:], ones_row_b[:],
                                rowb[:, nj * 512:(nj + 1) * 512],
                                start=True, stop=True)
                            nc.vector.tensor_copy(dst[:, nj * 512:(nj + 1) * 512],
                                                  ps[:])

                    # og = K^T W5 (pieces) and M update
                    for it in range(DP):
                        for nj in range(2):
                            sl = slice(nj * 512, (nj + 1) * 512)
                            ps = mmps.tile([128, 512], F32, tag="mm", name="mm")
                            nc.tensor.matmul(ps[:],
                                             ctxN[0][:, it * 128:(it + 1) * 128],
                                             W5b[0][:, sl], start=True, stop=False)
                            nc.tensor.matmul(ps[:],
                                             ctxN[1][:, it * 128:(it + 1) * 128],
                                             W5b[1][:, sl], start=False, stop=True)
                            t5 = p2.tile([128, 512], F32, tag="t5", name="t5")
                            nc.vector.tensor_tensor(t5[:], ps[:], ebc[:, sl],
                                                    OP.mult)
                            nc.vector.tensor_tensor(M_sb[:, it, sl],
                                                    M_sb[:, it, sl],
                                                    abc[:, sl], OP.mult)
                            nc.vector.tensor_tensor(M_sb[:, it, sl],
                                                    M_sb[:, it, sl], t5[:],
                                                    OP.subtract)
                        nc.vector.tensor_copy(M_bf[:, it, :], M_sb[:, it, :])

                    # c_out + fused epilogue
                    co = p2.tile([128, DP, CHUNK], BF16, tag="co", name="co")
                    for mj in range(DP):
                        ps = mmps.tile([128, 512], F32, tag="mm", name="mm")[:, :CHUNK]
                        for dp in range(DP):
                            nc.tensor.matmul(ps,
                                             M_bf[:, dp, mj * 128:(mj + 1) * 128],
                                             qch[:, dp, :],
                                             start=(dp == 0), stop=(dp == DP - 1))
                        nc.vector.tensor_copy(co[:, mj, :], ps)
                    csq = p2.tile([128, DP, CHUNK], BF16, tag="csq", name="csq")
                    nc.scalar.activation(csq[:], co[:], AF.Square)
                    ssps = aux.tile([128, 512], F32, tag="aux", name="aux")[:1, :CHUNK]
                    for dp in range(DP):
                        nc.tensor.matmul(ssps, ones_col_b[:], csq[:, dp, :],
                                         start=(dp == 0), stop=(dp == DP - 1))
                    oinv = p2.tile([1, CHUNK], F32, tag="oinv", name="oinv")
                    nc.scalar.activation(oinv[:], ssps, AF.Sqrt,
                                         bias=eps_col[:1, :], scale=1.0 / D)
                    nc.vector.reciprocal(oinv[:], oinv[:])
                    oinvb = p2.tile([1, CHUNK], BF16, tag="oinvb", name="oinvb")
                    nc.vector.tensor_copy(oinvb[:], oinv[:])
                    oibc = aux.tile([128, 512], F32, tag="aux", name="aux")[:, :CHUNK]
                    nc.tensor.matmul(oibc, ones_row_b[:], oinvb[:],
                                     start=True, stop=True)
                    oout = p2.tile([128, DP, CHUNK], F32, tag="oout", name="oout")
                    for dp in range(DP):
                        t6 = p2.tile([128, CHUNK], F32, tag="t6", name="t6")
                        nc.vector.tensor_tensor(t6[:], co[:, dp, :], oibc,
                                                OP.mult)
                        nc.vector.tensor_tensor(oout[:, dp, :], t6[:],
                                                bgch[:, dp, :], OP.mult)
                    nc.sync.dma_start(outT3[:, :, c0:c0 + CHUNK], oout[:])

                nc.sync.dma_start(Mo3[:], M_sb[:])
    nc.compile()
    return nc


def _get_nc():
    if "nc" not in _NC_CACHE:
        _NC_CACHE["nc"] = build()
    return _NC_CACHE["nc"]


def kernel(x, mem_state, buf_k, buf_v, norm_in_w, norm_kq_w, norm_out_w,
           Wk, Wq, Wv, Wg, Wb, conv_k_w, conv_k_b, conv_q_w, conv_q_b):
    x = np.asarray(x, np.float32)
    bf = lambda a: np.ascontiguousarray(np.asarray(a, np.float32)).astype(ml_dtypes.bfloat16)
    f32 = lambda a: np.ascontiguousarray(np.asarray(a, np.float32))

    ckw = np.asarray(conv_k_w, np.float32)
    cqw = np.asarray(conv_q_w, np.float32)
    WkTs = [bf(np.asarray(Wk).T * ckw[:, 0, r][None, :]) for r in range(3)]
    WqTs = [bf(np.asarray(Wq).T * cqw[:, 0, r][None, :]) for r in range(3)]
    shared = {
        "WkT0": WkTs[0], "WkT1": WkTs[1], "WkT2": WkTs[2],
        "WqT0": WqTs[0], "WqT1": WqTs[1], "WqT2": WqTs[2],
        "WvT": bf(np.asarray(Wv).T), "WgT": bf(np.asarray(Wg).T),
        "WbT": bf(np.asarray(Wb).T),
        "conv_k_b": f32(conv_k_b)[:, None], "conv_q_b": f32(conv_q_b)[:, None],
        "norm_in_w": f32(norm_in_w)[:, None],
        "norm_kq_w": f32(norm_kq_w)[:, None],
        "norm_out_w": f32(norm_out_w)[:, None],
    }
    in_maps = []
    for c in range(8):
        b = c % B
        m = dict(shared)
        m["xT"] = bf(np.ascontiguousarray(x[b].T))
        m["M0"] = f32(mem_state[b])
        m["buf_kT"] = bf(np.asarray(buf_k)[b].T)
        m["buf_v"] = bf(np.asarray(buf_v)[b])
        in_maps.append(m)

    nc = _get_nc()
    if os.environ.get("ATLAS_TRACE"):
        try:
            r = run_bass_kernel_spmd(nc, in_maps, list(range(8)), trace=True)
            globals()["LAST_EXEC_NS"] = r.exec_time_ns
            res = r.results
        except (ImportError, ModuleNotFoundError):
            import time as _t
            res = run_bass_kernel_spmd(nc, in_maps, list(range(8))).results
            t0 = _t.time()
            res = run_bass_kernel_spmd(nc, in_maps, list(range(8))).results
            globals()["LAST_EXEC_NS"] = int((_t.time() - t0) * 1e9)
    else:
        res = run_bass_kernel_spmd(nc, in_maps, list(range(8))).results

    out = np.stack([res[b]["outT"].T for b in range(B)])
    M = np.stack([res[b]["M_out"] for b in range(B)])
    bk = np.stack([res[b]["bkT"].T for b in range(B)])
    bv = np.stack([res[b]["bv"] for b in range(B)])
    return (out.astype(np.float32), M.astype(np.float32),
            bk.astype(np.float32), bv.astype(np.float32))
